# revision 32
# baseline (speedup 1.0000x reference)
"""GQA (16 q-heads / 4 kv-heads, D=128, S=2048, E=2048, B=2) on 8 trn2 cores.

Sharding: core = 4*b + g  (b in {0,1} batch, g in {0..3} kv-head group).
Each core computes its batch's 4 query heads (one kv group) end-to-end and
the host sums the 4 partial o_proj outputs per batch.

v3 (single interleaved program, engines balanced, DMA-consolidated):
  - Blocked DRAM layouts + AP.transpose give one DMA per weight tensor and
    4 DMAs per x position-chunk (~65 DMAs total vs ~250).
  - Phase A per 512-position chunk: K proj+RoPE, Q(h0) proj+RoPE, V proj +
    PE-transpose to natural bf16, Q(h1..h3) proj+RoPE.  RoPE rotate-half is
    a signed-permutation matmul on PE (no SBUF swap DMAs); cos/sin mults
    split DVE/DVE, add on Pool.
  - Attention in four 512-wide query chunks: scoresT per sk-tile in f32r,
    Exp on Act straight from PSUM to bf16 SBUF, bf16 AV matmuls (V-natural
    stationary), softmax denominator via two bf16 chain-adds (DVE + Pool),
    ones-matmul partition reduce, reciprocal, Pool partition_broadcast,
    DVE normalize multiply into f32r ot.
  - o_proj of chunk c interleaved into attention of chunk c+1; stores
    staged through SBUF [128,1024] tiles via DVE/Act copies.
"""

import numpy as np
import ml_dtypes

import concourse.bass as bass
import concourse.bacc as bacc
import concourse.mybir as mybir
import concourse.tile as tile
from concourse.bass_utils import run_bass_kernel_spmd

B, S, E = 2, 2048, 2048
H, HKV, D = 16, 4, 128
G = H // HKV          # 4 query heads per kv group
GD = G * D            # 512 channels per group
NCORES = 8
SCALE = 1.0 / float(np.sqrt(D))
ROPE_BASE = 10000.0

NE = E // 128         # 16 e-blocks (contraction for projections)
NC4 = S // 512        # 4 position chunks of 512 (projection granularity)
NST = S // 128        # 16 sk-tiles of 128
CHS = [512, 1024, 512]          # attention query-chunk widths
COFF = [0, 512, 1536]           # their offsets
CHMAX = 1024

F32 = mybir.dt.float32
F32R = mybir.dt.float32r
BF16 = mybir.dt.bfloat16
AF = mybir.ActivationFunctionType
OP = mybir.AluOpType


def _r(ap):
    return ap.bitcast(F32R)


def _emit(nc, tc, xTb, wqb, wkb, wvb, wob, cosT, sinT, rotP, ident, onesb, out):
    from contextlib import ExitStack
    es = ExitStack()
    with es:
        cpool = es.enter_context(tc.tile_pool(name="const", bufs=1))
        wopool = es.enter_context(tc.tile_pool(name="wo", bufs=2))
        xpool = es.enter_context(tc.tile_pool(name="xs", bufs=9))
        rpool = es.enter_context(tc.tile_pool(name="rope", bufs=2))
        etpool = es.enter_context(tc.tile_pool(name="et", bufs=5))
        bcspool = es.enter_context(tc.tile_pool(name="bcs", bufs=1))
        dnpool = es.enter_context(tc.tile_pool(name="dn", bufs=2))
        rcpool = es.enter_context(tc.tile_pool(name="rc", bufs=1))
        otpool = es.enter_context(tc.tile_pool(name="ot", bufs=6))
        ostgpool = es.enter_context(tc.tile_pool(name="ostg", bufs=3))
        vtpool = es.enter_context(tc.tile_pool(name="vt", bufs=2))
        pssc = es.enter_context(
            tc.tile_pool(name="pssc", bufs=2, space=bass.MemorySpace.PSUM))
        psav = es.enter_context(
            tc.tile_pool(name="psav", bufs=1, space=bass.MemorySpace.PSUM))
        psmx = es.enter_context(
            tc.tile_pool(name="psmx", bufs=2, space=bass.MemorySpace.PSUM))

        # ---- small constants ----
        id_sb = cpool.tile([128, 128], F32, tag="id")
        ones_sb = cpool.tile([128, 128], BF16, tag="ones")
        rp_sb = cpool.tile([128, 128], F32R, tag="rp")
        nc.sync.dma_start(out=id_sb[:], in_=ident.ap())
        nc.sync.dma_start(out=ones_sb[:], in_=onesb.ap())
        nc.sync.dma_start(out=rp_sb[:], in_=rotP.ap().bitcast(F32R))
        cos_sb = cpool.tile([D, S], BF16, tag="cos")
        sin_sb = cpool.tile([D, S], BF16, tag="sin")
        nc.sync.dma_start(out=cos_sb[:], in_=cosT.ap())
        nc.sync.dma_start(out=sin_sb[:], in_=sinT.ap())

        # ---- consolidated weight loads; wq/wv DMAs are issued after the
        # first x chunk so the critical path to the first K matmul is short
        wk_t = cpool.tile([128, NE, D], F32R, tag="wkt")
        nc.sync.dma_start(out=wk_t[:],
                          in_=wkb.ap().transpose([1, 0, 2]).bitcast(F32R))
        wq_t = cpool.tile([128, NE, GD], F32R, tag="wqt")
        wv_t = cpool.tile([128, NE, D], F32R, tag="wvt")

        # ---- persistent activations (bf16: same PE rate, half SBUF) ----
        kt = cpool.tile([D, S], BF16, tag="kt")
        qt = [cpool.tile([D, S], BF16, tag=f"qt{h}", name=f"qt{h}")
              for h in range(G)]
        vn = cpool.tile([128, NST, D], BF16, tag="vn")

        def rope(dst, ps, sl):
            # rotate_half as a signed-permutation matmul; then
            # dst = q*cos + rot(q)*sin.  The rot output borrows a scores-pool
            # PSUM slot (idle during phase A) to keep psmx free for the next
            # projection.
            qraw = rpool.tile([128, 512], F32R, tag="qraw")
            tmc = rpool.tile([128, 512], F32, tag="tmc")
            t2 = rpool.tile([128, 512], F32, tag="t2")
            nc.scalar.copy(qraw[:], ps[:])
            rot = pssc.tile([128, 512], F32, tag="sc", name="rot")
            nc.tensor.matmul(rot[:], rp_sb[:], qraw[:], start=True, stop=True)
            nc.vector.tensor_tensor(tmc[:], qraw[:], cos_sb[:, sl], OP.mult)
            nc.vector.tensor_tensor(t2[:], rot[:], sin_sb[:, sl], OP.mult)
            nc.gpsimd.tensor_tensor(dst, tmc[:], t2[:], OP.add)

        def load_x(c4):
            sl = slice(c4 * 512, (c4 + 1) * 512)
            xsl = [xpool.tile([128, 2, 512], F32R, tag="xs",
                              name=f"xs{c4}_{jj}") for jj in range(8)]
            for jj in range(8):
                nc.sync.dma_start(
                    out=xsl[jj][:],
                    in_=xTb.ap()[jj * 2:(jj + 1) * 2, :, sl]
                        .transpose([1, 0, 2]).bitcast(F32R))
            return xsl

        def proj(wt, cslice, xsl):
            ps = psmx.tile([128, 512], F32, tag="mx", name="ps")
            for j in range(NE):
                nc.tensor.matmul(ps[:], wt[:, j, cslice],
                                 xsl[j // 2][:, j % 2, :],
                                 start=(j == 0), stop=(j == NE - 1))
            return ps

        def qproj(h, c4, xsl):
            sl = slice(c4 * 512, (c4 + 1) * 512)
            ps = proj(wq_t, slice(h * D, (h + 1) * D), xsl)
            rope(qt[h][:, sl], ps, sl)

        # ============ phase A: K/V for all chunks, Q for chunk 0 ============
        # Remaining Q projections are interleaved into the act-gated
        # attention windows below (x slices reloaded through the same pool).
        for c4 in range(NC4):
            sl = slice(c4 * 512, (c4 + 1) * 512)
            xsl = load_x(c4)
            if c4 == 0:
                for jj in range(4):
                    nc.sync.dma_start(
                        out=wq_t[:, jj * 4:(jj + 1) * 4, :],
                        in_=wqb.ap()[jj * 4:(jj + 1) * 4, :, :]
                            .transpose([1, 0, 2]).bitcast(F32R))
                nc.sync.dma_start(
                    out=wv_t[:],
                    in_=wvb.ap().transpose([1, 0, 2]).bitcast(F32R))
            # K
            ps = proj(wk_t, slice(0, D), xsl)
            rope(kt[:, sl], ps, sl)
            # V -> natural bf16 via PE transpose
            ps = proj(wv_t, slice(0, D), xsl)
            vt = vtpool.tile([D, 512], F32, tag="vt")
            nc.scalar.copy(vt[:], ps[:])
            for tt in range(4):
                trp = psmx.tile([128, 128], F32, tag="mx", name="trp")
                nc.tensor.transpose(trp[:], vt[:, tt * 128:(tt + 1) * 128],
                                    id_sb[:])
                nc.vector.tensor_copy(vn[:, c4 * 4 + tt, :], trp[:])
            if c4 == 0:
                for h in range(G):
                    qproj(h, 0, xsl)

        # ================= phases B+C: attention + o_proj =================
        def attn_head(off, chw, h):
            """Scores/exp/AV/denominator/normalize for a chw-wide query chunk
            at offset off, head h.  Returns the normalized [D, chw] f32r ot."""
            nhf = chw // 512
            dn = dnpool.tile([128, chw], BF16, tag="dn")
            av = psav.tile([D, chw], F32, tag="av")
            et_first = None
            for t in range(NST):
                sc = pssc.tile([128, chw], F32, tag="sc")
                for hf in range(nhf):
                    qsl = slice(off + hf * 512, off + (hf + 1) * 512)
                    nc.tensor.matmul(sc[:, hf * 512:(hf + 1) * 512],
                                     kt[:, t * 128:(t + 1) * 128],
                                     qt[h][:, qsl], start=True, stop=True)
                et = etpool.tile([128, chw], BF16, tag="et")
                nc.scalar.activation(et[:], sc[:], AF.Exp, scale=SCALE)
                for hf in range(nhf):
                    nc.tensor.matmul(av[:, hf * 512:(hf + 1) * 512],
                                     vn[:, t, :],
                                     et[:, hf * 512:(hf + 1) * 512],
                                     start=(t == 0), stop=(t == NST - 1))
                # denominator: single DVE chain, always caught up with exp
                if t == 0:
                    et_first = et
                elif t == 1:
                    nc.vector.tensor_tensor(dn[:], et_first[:], et[:], OP.add)
                else:
                    nc.vector.tensor_tensor(dn[:], dn[:], et[:], OP.add)
            rc = rcpool.tile([1, chw], BF16, tag="rc")
            for hf in range(nhf):
                sm = psmx.tile([1, 512], F32, tag="mx", name="sm")
                nc.tensor.matmul(sm[:], ones_sb[:, 0:1],
                                 dn[:, hf * 512:(hf + 1) * 512],
                                 start=True, stop=True)
                with nc.allow_low_precision(reason="bf16 softmax denom recip"):
                    nc.vector.reciprocal(rc[:, hf * 512:(hf + 1) * 512], sm[:])
            bcs = bcspool.tile([128, chw], BF16, tag="bcs")
            nc.gpsimd.partition_broadcast(bcs[:], rc[:])
            ot = otpool.tile([D, chw], F32R, tag="ot")
            nc.vector.tensor_tensor(ot[:], av[:], bcs[:], OP.mult)
            return ot

        def oproj_pair(off, chw, eo0, ots, ci, copy_eng):
            """Two adjacent eo column-groups (1024 cols of E) of o_proj for
            the chunk at offset off: one [128,1024] staging tile per s-tile,
            one store each."""
            wots = []
            for eo in (eo0, eo0 + 1):
                wot = wopool.tile([128, G, 512], F32R, tag="wo",
                                  name=f"wo{ci}_{eo}")
                nc.sync.dma_start(
                    out=wot[:],
                    in_=wob.ap()[:, :, eo * 512:(eo + 1) * 512]
                        .transpose([1, 0, 2]).bitcast(F32R))
                wots.append(wot)
            for st in range(chw // 128):
                ostg = ostgpool.tile([128, 1024], F32, tag="ostg",
                                     name=f"ostg{ci}_{eo0}_{st}")
                for k in range(2):
                    op = psmx.tile([128, 512], F32, tag="mx", name="op")
                    for h in range(G):
                        nc.tensor.matmul(op[:],
                                         ots[h][:, st * 128:(st + 1) * 128],
                                         wots[k][:, h, :],
                                         start=(h == 0), stop=(h == G - 1))
                    dst = ostg[:, k * 512:(k + 1) * 512]
                    if copy_eng == 'act':
                        nc.scalar.copy(dst, op[:])
                    else:
                        nc.vector.tensor_copy(dst, op[:])
                nc.sync.dma_start(
                    out=out.ap()[off + st * 128:off + (st + 1) * 128,
                                 eo0 * 512:(eo0 + 2) * 512],
                    in_=ostg[:])

        # B0 (512-wide): deferred Q projections for position chunks 1 and 2
        # fill the act-gated PE gaps.
        ots0, ots1, ots2 = [], [], []
        xs1 = xs2 = xs3 = None
        for h in range(G):
            ots0.append(attn_head(COFF[0], CHS[0], h))
            if h == 0:
                xs1 = load_x(1)
                qproj(0, 1, xs1)
                qproj(1, 1, xs1)
            elif h == 1:
                qproj(2, 1, xs1)
                qproj(3, 1, xs1)
            elif h == 2:
                xs2 = load_x(2)
                qproj(0, 2, xs2)
                qproj(1, 2, xs2)
            else:
                qproj(2, 2, xs2)
                qproj(3, 2, xs2)
        # B1 (1024-wide): o_proj of chunk 0 + Q projections for chunk 3
        for h in range(G):
            ots1.append(attn_head(COFF[1], CHS[1], h))
            if h == 0:
                oproj_pair(COFF[0], CHS[0], 0, ots0, 0, 'dve')
                xs3 = load_x(3)
                qproj(0, 3, xs3)
                qproj(1, 3, xs3)
            elif h == 1:
                oproj_pair(COFF[0], CHS[0], 2, ots0, 0, 'dve')
                qproj(2, 3, xs3)
                qproj(3, 3, xs3)
        # B2 (512-wide): o_proj of chunk 1
        for h in range(G):
            ots2.append(attn_head(COFF[2], CHS[2], h))
            if h == 0:
                oproj_pair(COFF[1], CHS[1], 0, ots1, 1, 'dve')
            elif h == 1:
                oproj_pair(COFF[1], CHS[1], 2, ots1, 1, 'dve')
        # tail: o_proj of chunk 2, staging copies on the now-idle Act engine
        oproj_pair(COFF[2], CHS[2], 0, ots2, 2, 'act')
        oproj_pair(COFF[2], CHS[2], 2, ots2, 2, 'act')


def _build():
    nc = bacc.Bacc("TRN2", target_bir_lowering=False, debug=False,
                   num_devices=NCORES)
    xTb = nc.dram_tensor("xTb", [NE, 128, S], F32, kind="ExternalInput")
    wqb = nc.dram_tensor("wqb", [NE, 128, GD], F32, kind="ExternalInput")
    wkb = nc.dram_tensor("wkb", [NE, 128, D], F32, kind="ExternalInput")
    wvb = nc.dram_tensor("wvb", [NE, 128, D], F32, kind="ExternalInput")
    wob = nc.dram_tensor("wob", [G, 128, E], F32, kind="ExternalInput")
    cosT = nc.dram_tensor("cosT", [D, S], BF16, kind="ExternalInput")
    sinT = nc.dram_tensor("sinT", [D, S], BF16, kind="ExternalInput")
    rotP = nc.dram_tensor("rotP", [128, 128], F32, kind="ExternalInput")
    ident = nc.dram_tensor("ident", [128, 128], F32, kind="ExternalInput")
    onesb = nc.dram_tensor("onesb", [128, 128], BF16, kind="ExternalInput")
    out = nc.dram_tensor("out", [S, E], F32, kind="ExternalOutput")
    with tile.TileContext(nc) as tc:
        _emit(nc, tc, xTb, wqb, wkb, wvb, wob, cosT, sinT, rotP, ident,
              onesb, out)
    nc.compile()
    return nc


def _rope_tables():
    inv = 1.0 / (ROPE_BASE ** (np.arange(0, D, 2, dtype=np.float64) / D))
    t = np.arange(S, dtype=np.float64)
    freqs = t[:, None] * inv[None, :]                    # [S, D/2]
    emb = np.concatenate([freqs, freqs], axis=-1)        # [S, D]
    cosT = np.cos(emb).T.astype(ml_dtypes.bfloat16)      # [D, S]
    sinT = np.sin(emb).T.astype(ml_dtypes.bfloat16)
    return np.ascontiguousarray(cosT), np.ascontiguousarray(sinT)


def _rot_perm():
    # rot(q)[d] = -q[d+64] for d<64, +q[d-64] for d>=64, as a stationary
    # matmul operand: rot = P^T @ q with P[k, m] below.
    p = np.zeros((128, 128), dtype=np.float32)
    for d in range(64):
        p[d + 64, d] = -1.0
        p[d, d + 64] = 1.0
    return p


_NC = None
LAST_RESULTS = None


def kernel(hidden_states, wq, wk, wv, wo):
    global _NC, LAST_RESULTS
    if _NC is None:
        _NC = _build()
    cosT, sinT = _rope_tables()
    ident = np.eye(128, dtype=np.float32)
    onesb = np.ones((128, 128), dtype=ml_dtypes.bfloat16)
    rotP = _rot_perm()
    hs = np.asarray(hidden_states, dtype=np.float32)
    wq = np.asarray(wq, dtype=np.float32)
    wk = np.asarray(wk, dtype=np.float32)
    wv = np.asarray(wv, dtype=np.float32)
    wo = np.asarray(wo, dtype=np.float32)

    in_maps = []
    for core in range(NCORES):
        b, g = divmod(core, G)
        in_maps.append({
            "xTb": np.ascontiguousarray(hs[b].T).reshape(NE, 128, S),
            "wqb": np.ascontiguousarray(
                wq[:, GD * g:GD * (g + 1)]).reshape(NE, 128, GD),
            "wkb": np.ascontiguousarray(
                wk[:, D * g:D * (g + 1)]).reshape(NE, 128, D),
            "wvb": np.ascontiguousarray(
                wv[:, D * g:D * (g + 1)]).reshape(NE, 128, D),
            "wob": np.ascontiguousarray(
                wo[GD * g:GD * (g + 1), :]).reshape(G, 128, E),
            "cosT": cosT,
            "sinT": sinT,
            "rotP": rotP,
            "ident": ident,
            "onesb": onesb,
        })

    res = run_bass_kernel_spmd(_NC, in_maps, list(range(NCORES)))
    LAST_RESULTS = res
    outs = [np.asarray(res.results[i]["out"], dtype=np.float32)
            for i in range(NCORES)]
    full = np.stack([sum(outs[b * G:(b + 1) * G]) for b in range(B)], axis=0)
    return full.astype(np.float32)


# revision 38
# speedup vs baseline: 1.0680x; 1.0680x over previous
"""GQA (16 q-heads / 4 kv-heads, D=128, S=2048, E=2048, B=2) on 8 trn2 cores.

Sharding: core = 4*b + g  (b in {0,1} batch, g in {0..3} kv-head group).
Each core computes its batch's 4 query heads (one kv group) end-to-end and
the host sums the 4 partial o_proj outputs per batch.

v3 (single interleaved program, engines balanced, DMA-consolidated):
  - Blocked DRAM layouts + AP.transpose give one DMA per weight tensor and
    4 DMAs per x position-chunk (~65 DMAs total vs ~250).
  - Phase A per 512-position chunk: K proj+RoPE, Q(h0) proj+RoPE, V proj +
    PE-transpose to natural bf16, Q(h1..h3) proj+RoPE.  RoPE rotate-half is
    a signed-permutation matmul on PE (no SBUF swap DMAs); cos/sin mults
    split DVE/DVE, add on Pool.
  - Attention in four 512-wide query chunks: scoresT per sk-tile in f32r,
    Exp on Act straight from PSUM to bf16 SBUF, bf16 AV matmuls (V-natural
    stationary), softmax denominator via two bf16 chain-adds (DVE + Pool),
    ones-matmul partition reduce, reciprocal, Pool partition_broadcast,
    DVE normalize multiply into f32r ot.
  - o_proj of chunk c interleaved into attention of chunk c+1; stores
    staged through SBUF [128,1024] tiles via DVE/Act copies.
"""

import numpy as np
import ml_dtypes

import concourse.bass as bass
import concourse.bacc as bacc
import concourse.mybir as mybir
import concourse.tile as tile
from concourse.bass_utils import run_bass_kernel_spmd

B, S, E = 2, 2048, 2048
H, HKV, D = 16, 4, 128
G = H // HKV          # 4 query heads per kv group
GD = G * D            # 512 channels per group
NCORES = 8
SCALE = 1.0 / float(np.sqrt(D))
ROPE_BASE = 10000.0

NE = E // 128         # 16 e-blocks (contraction for projections)
NC4 = S // 512        # 4 position chunks of 512 (projection granularity)
NST = S // 128        # 16 sk-tiles of 128
CHS = [512, 1024, 512]          # attention query-chunk widths
COFF = [0, 512, 1536]           # their offsets
CHMAX = 1024

F32 = mybir.dt.float32
F32R = mybir.dt.float32r
BF16 = mybir.dt.bfloat16
AF = mybir.ActivationFunctionType
OP = mybir.AluOpType


def _r(ap):
    return ap.bitcast(F32R)


def _emit(nc, tc, xTb, wqb, wkb, wvb, wob, cosT, sinT, rotP, ident, onesb, out):
    from contextlib import ExitStack
    es = ExitStack()
    with es:
        cpool = es.enter_context(tc.tile_pool(name="const", bufs=1))
        wopool = es.enter_context(tc.tile_pool(name="wo", bufs=2))
        xpool = es.enter_context(tc.tile_pool(name="xs", bufs=9))
        rpool = es.enter_context(tc.tile_pool(name="rope", bufs=2))
        etpool = es.enter_context(tc.tile_pool(name="et", bufs=5))
        bcspool = es.enter_context(tc.tile_pool(name="bcs", bufs=1))
        dnpool = es.enter_context(tc.tile_pool(name="dn", bufs=2))
        rcpool = es.enter_context(tc.tile_pool(name="rc", bufs=1))
        otpool = es.enter_context(tc.tile_pool(name="ot", bufs=6))
        ostgpool = es.enter_context(tc.tile_pool(name="ostg", bufs=3))
        vtpool = es.enter_context(tc.tile_pool(name="vt", bufs=2))
        pssc = es.enter_context(
            tc.tile_pool(name="pssc", bufs=2, space=bass.MemorySpace.PSUM))
        psav = es.enter_context(
            tc.tile_pool(name="psav", bufs=1, space=bass.MemorySpace.PSUM))
        psmx = es.enter_context(
            tc.tile_pool(name="psmx", bufs=2, space=bass.MemorySpace.PSUM))

        # ---- small constants ----
        id_sb = cpool.tile([128, 128], F32, tag="id")
        ones_sb = cpool.tile([128, 128], BF16, tag="ones")
        rp_sb = cpool.tile([128, 128], F32R, tag="rp")
        nc.sync.dma_start(out=id_sb[:], in_=ident.ap())
        nc.sync.dma_start(out=ones_sb[:], in_=onesb.ap())
        nc.sync.dma_start(out=rp_sb[:], in_=rotP.ap().bitcast(F32R))
        cos_sb = cpool.tile([D, S], BF16, tag="cos")
        sin_sb = cpool.tile([D, S], BF16, tag="sin")
        nc.sync.dma_start(out=cos_sb[:], in_=cosT.ap())
        nc.sync.dma_start(out=sin_sb[:], in_=sinT.ap())

        # ---- consolidated weight loads; wq/wv DMAs are issued after the
        # first x chunk so the critical path to the first K matmul is short
        wk_t = cpool.tile([128, NE, D], F32R, tag="wkt")
        nc.sync.dma_start(out=wk_t[:],
                          in_=wkb.ap().transpose([1, 0, 2]).bitcast(F32R))
        wq_t = cpool.tile([128, NE, GD], F32R, tag="wqt")
        wv_t = cpool.tile([128, NE, D], F32R, tag="wvt")

        # ---- persistent activations (bf16: same PE rate, half SBUF) ----
        kt = cpool.tile([D, S], BF16, tag="kt")
        qt = [cpool.tile([D, S], BF16, tag=f"qt{h}", name=f"qt{h}")
              for h in range(G)]
        vn = cpool.tile([128, NST, D], BF16, tag="vn")

        def rope(dst, ps, sl, in_attn=False):
            # rotate_half as a signed-permutation matmul; then
            # dst = q*cos + rot(q)*sin.  During phase A the qraw copy runs on
            # the idle Act engine and rot borrows an idle scores-pool PSUM
            # slot; inside attention windows both would collide with exp /
            # scores, so qraw moves to DVE and rot stays in psmx.
            qraw = rpool.tile([128, 512], F32R, tag="qraw")
            tmc = rpool.tile([128, 512], F32, tag="tmc")
            t2 = rpool.tile([128, 512], F32, tag="t2")
            if in_attn:
                nc.vector.tensor_copy(qraw[:], ps[:])
                rot = psmx.tile([128, 512], F32, tag="mx", name="rot")
            else:
                nc.scalar.copy(qraw[:], ps[:])
                rot = pssc.tile([128, 512], F32, tag="sc", name="rot")
            nc.tensor.matmul(rot[:], rp_sb[:], qraw[:], start=True, stop=True)
            nc.vector.tensor_tensor(tmc[:], qraw[:], cos_sb[:, sl], OP.mult)
            nc.vector.tensor_tensor(t2[:], rot[:], sin_sb[:, sl], OP.mult)
            nc.gpsimd.tensor_tensor(dst, tmc[:], t2[:], OP.add)

        def load_x(c4):
            sl = slice(c4 * 512, (c4 + 1) * 512)
            xsl = [xpool.tile([128, 2, 512], F32R, tag="xs",
                              name=f"xs{c4}_{jj}") for jj in range(8)]
            for jj in range(8):
                nc.sync.dma_start(
                    out=xsl[jj][:],
                    in_=xTb.ap()[jj * 2:(jj + 1) * 2, :, sl]
                        .transpose([1, 0, 2]).bitcast(F32R))
            return xsl

        def proj(wt, cslice, xsl):
            ps = psmx.tile([128, 512], F32, tag="mx", name="ps")
            for j in range(NE):
                nc.tensor.matmul(ps[:], wt[:, j, cslice],
                                 xsl[j // 2][:, j % 2, :],
                                 start=(j == 0), stop=(j == NE - 1))
            return ps

        def qproj(h, c4, xsl):
            sl = slice(c4 * 512, (c4 + 1) * 512)
            ps = proj(wq_t, slice(h * D, (h + 1) * D), xsl)
            rope(qt[h][:, sl], ps, sl)

        # ============ phase A: K/V for all chunks, Q for chunk 0 ============
        # Remaining Q projections are interleaved into the act-gated
        # attention windows below (x slices reloaded through the same pool).
        for c4 in range(NC4):
            sl = slice(c4 * 512, (c4 + 1) * 512)
            xsl = load_x(c4)
            if c4 == 0:
                for jj in range(4):
                    nc.sync.dma_start(
                        out=wq_t[:, jj * 4:(jj + 1) * 4, :],
                        in_=wqb.ap()[jj * 4:(jj + 1) * 4, :, :]
                            .transpose([1, 0, 2]).bitcast(F32R))
                nc.sync.dma_start(
                    out=wv_t[:],
                    in_=wvb.ap().transpose([1, 0, 2]).bitcast(F32R))
            # K
            ps = proj(wk_t, slice(0, D), xsl)
            rope(kt[:, sl], ps, sl)
            # V -> natural bf16 via PE transpose
            ps = proj(wv_t, slice(0, D), xsl)
            vt = vtpool.tile([D, 512], F32, tag="vt")
            nc.scalar.copy(vt[:], ps[:])
            for tt in range(4):
                trp = psmx.tile([128, 128], F32, tag="mx", name="trp")
                nc.tensor.transpose(trp[:], vt[:, tt * 128:(tt + 1) * 128],
                                    id_sb[:])
                nc.vector.tensor_copy(vn[:, c4 * 4 + tt, :], trp[:])
            if c4 == 0:
                for h in range(G):
                    qproj(h, 0, xsl)

        # ================= phases B+C: attention + o_proj =================
        # Deferred PE work (remaining Q projections, previous-chunk o_proj)
        # is queued as ~1-matmul units and drained a couple per t-iteration,
        # so the Act engine never sees a multi-microsecond scores gap.
        import collections
        fill = collections.deque()

        def drain(n):
            while n > 0 and fill:
                fill.popleft()()
                n -= 1

        def enqueue_qproj(h, c4, xsl):
            sl = slice(c4 * 512, (c4 + 1) * 512)
            state = {}

            def mk(j):
                def unit():
                    if j == 0:
                        state['ps'] = psmx.tile([128, 512], F32, tag="mx",
                                                name="ps")
                    ps = state['ps']
                    for jj in (2 * j, 2 * j + 1):
                        nc.tensor.matmul(ps[:], wq_t[:, jj, h * D:(h + 1) * D],
                                         xsl[jj // 2][:, jj % 2, :],
                                         start=(jj == 0), stop=(jj == NE - 1))
                return unit
            for j in range(NE // 2):
                fill.append(mk(j))
            fill.append(lambda: rope(qt[h][:, sl], state['ps'], sl,
                                     in_attn=True))

        def attn_head(off, chw, h):
            """Scores/exp/AV/denominator/normalize for a chw-wide query chunk
            at offset off, head h.  Returns the normalized [D, chw] f32r ot."""
            nhf = chw // 512
            dn = dnpool.tile([128, chw], BF16, tag="dn")
            av = psav.tile([D, chw], F32, tag="av")
            et_first = None
            for t in range(NST):
                sc = pssc.tile([128, chw], F32, tag="sc")
                for hf in range(nhf):
                    qsl = slice(off + hf * 512, off + (hf + 1) * 512)
                    nc.tensor.matmul(sc[:, hf * 512:(hf + 1) * 512],
                                     kt[:, t * 128:(t + 1) * 128],
                                     qt[h][:, qsl], start=True, stop=True)
                et = etpool.tile([128, chw], BF16, tag="et")
                nc.scalar.activation(et[:], sc[:], AF.Exp, scale=SCALE)
                for hf in range(nhf):
                    nc.tensor.matmul(av[:, hf * 512:(hf + 1) * 512],
                                     vn[:, t, :],
                                     et[:, hf * 512:(hf + 1) * 512],
                                     start=(t == 0), stop=(t == NST - 1))
                # denominator: single DVE chain, always caught up with exp
                if t == 0:
                    et_first = et
                elif t == 1:
                    nc.vector.tensor_tensor(dn[:], et_first[:], et[:], OP.add)
                else:
                    nc.vector.tensor_tensor(dn[:], dn[:], et[:], OP.add)
                if t >= 2:
                    drain(2)
            rc = rcpool.tile([1, chw], BF16, tag="rc")
            for hf in range(nhf):
                sm = psmx.tile([1, 512], F32, tag="mx", name="sm")
                nc.tensor.matmul(sm[:], ones_sb[:, 0:1],
                                 dn[:, hf * 512:(hf + 1) * 512],
                                 start=True, stop=True)
                with nc.allow_low_precision(reason="bf16 softmax denom recip"):
                    nc.vector.reciprocal(rc[:, hf * 512:(hf + 1) * 512], sm[:])
            bcs = bcspool.tile([128, chw], BF16, tag="bcs")
            nc.gpsimd.partition_broadcast(bcs[:], rc[:])
            ot = otpool.tile([D, chw], F32R, tag="ot")
            nc.vector.tensor_tensor(ot[:], av[:], bcs[:], OP.mult)
            return ot

        def enqueue_oproj_pair(off, chw, eo0, ots, ci, copy_eng):
            """Two adjacent eo column-groups (1024 cols of E) of o_proj for
            the chunk at offset off: one [128,1024] staging tile per s-tile,
            one store each.  Each (st, k) group is one filler unit."""
            wots = []
            for eo in (eo0, eo0 + 1):
                wot = wopool.tile([128, G, 512], F32R, tag="wo",
                                  name=f"wo{ci}_{eo}")
                nc.sync.dma_start(
                    out=wot[:],
                    in_=wob.ap()[:, :, eo * 512:(eo + 1) * 512]
                        .transpose([1, 0, 2]).bitcast(F32R))
                wots.append(wot)
            for st in range(chw // 128):
                state = {}

                def mk(st, k):
                    def unit():
                        if k == 0:
                            state['ostg'] = ostgpool.tile(
                                [128, 1024], F32, tag="ostg",
                                name=f"ostg{ci}_{eo0}_{st}")
                        ostg = state['ostg']
                        op = psmx.tile([128, 512], F32, tag="mx", name="op")
                        for h in range(G):
                            nc.tensor.matmul(
                                op[:], ots[h][:, st * 128:(st + 1) * 128],
                                wots[k][:, h, :],
                                start=(h == 0), stop=(h == G - 1))
                        dst = ostg[:, k * 512:(k + 1) * 512]
                        if copy_eng == 'act':
                            nc.scalar.copy(dst, op[:])
                        else:
                            nc.vector.tensor_copy(dst, op[:])
                        if k == 1:
                            nc.sync.dma_start(
                                out=out.ap()[off + st * 128:
                                             off + (st + 1) * 128,
                                             eo0 * 512:(eo0 + 2) * 512],
                                in_=ostg[:])
                    return unit
                fill.append(mk(st, 0))
                fill.append(mk(st, 1))

        # B0 (512-wide): deferred Q projections for position chunks 1 and 2
        # drain into the PE slack of the attention loop.  x slices are
        # re-loaded one stage ahead so filler matmuls never wait on DMA.
        xs1 = load_x(1)
        ots0, ots1, ots2 = [], [], []
        xs2 = None
        for h in range(G):
            if h == 0:
                for hq in range(G):
                    enqueue_qproj(hq, 1, xs1)
            elif h == 1:
                xs2 = load_x(2)
            elif h == 2:
                for hq in range(G):
                    enqueue_qproj(hq, 2, xs2)
            ots0.append(attn_head(COFF[0], CHS[0], h))
            drain(8)
        drain(len(fill))
        # B1 (1024-wide): o_proj of chunk 0 + Q projections for chunk 3
        xs3 = load_x(3)
        for h in range(G):
            if h == 0:
                enqueue_oproj_pair(COFF[0], CHS[0], 0, ots0, 0, 'dve')
                for hq in range(G):
                    enqueue_qproj(hq, 3, xs3)
                enqueue_oproj_pair(COFF[0], CHS[0], 2, ots0, 0, 'dve')
            ots1.append(attn_head(COFF[1], CHS[1], h))
            drain(8)
        drain(len(fill))
        # B2 (512-wide): o_proj of chunk 1
        for h in range(G):
            if h == 0:
                enqueue_oproj_pair(COFF[1], CHS[1], 0, ots1, 1, 'dve')
                enqueue_oproj_pair(COFF[1], CHS[1], 2, ots1, 1, 'dve')
            ots2.append(attn_head(COFF[2], CHS[2], h))
            drain(8)
        drain(len(fill))
        # tail: o_proj of chunk 2, staging copies on the now-idle Act engine
        enqueue_oproj_pair(COFF[2], CHS[2], 0, ots2, 2, 'act')
        enqueue_oproj_pair(COFF[2], CHS[2], 2, ots2, 2, 'act')
        drain(len(fill))


def _build():
    nc = bacc.Bacc("TRN2", target_bir_lowering=False, debug=False,
                   num_devices=NCORES)
    xTb = nc.dram_tensor("xTb", [NE, 128, S], F32, kind="ExternalInput")
    wqb = nc.dram_tensor("wqb", [NE, 128, GD], F32, kind="ExternalInput")
    wkb = nc.dram_tensor("wkb", [NE, 128, D], F32, kind="ExternalInput")
    wvb = nc.dram_tensor("wvb", [NE, 128, D], F32, kind="ExternalInput")
    wob = nc.dram_tensor("wob", [G, 128, E], F32, kind="ExternalInput")
    cosT = nc.dram_tensor("cosT", [D, S], BF16, kind="ExternalInput")
    sinT = nc.dram_tensor("sinT", [D, S], BF16, kind="ExternalInput")
    rotP = nc.dram_tensor("rotP", [128, 128], F32, kind="ExternalInput")
    ident = nc.dram_tensor("ident", [128, 128], F32, kind="ExternalInput")
    onesb = nc.dram_tensor("onesb", [128, 128], BF16, kind="ExternalInput")
    out = nc.dram_tensor("out", [S, E], F32, kind="ExternalOutput")
    with tile.TileContext(nc) as tc:
        _emit(nc, tc, xTb, wqb, wkb, wvb, wob, cosT, sinT, rotP, ident,
              onesb, out)
    nc.compile()
    return nc


def _rope_tables():
    inv = 1.0 / (ROPE_BASE ** (np.arange(0, D, 2, dtype=np.float64) / D))
    t = np.arange(S, dtype=np.float64)
    freqs = t[:, None] * inv[None, :]                    # [S, D/2]
    emb = np.concatenate([freqs, freqs], axis=-1)        # [S, D]
    cosT = np.cos(emb).T.astype(ml_dtypes.bfloat16)      # [D, S]
    sinT = np.sin(emb).T.astype(ml_dtypes.bfloat16)
    return np.ascontiguousarray(cosT), np.ascontiguousarray(sinT)


def _rot_perm():
    # rot(q)[d] = -q[d+64] for d<64, +q[d-64] for d>=64, as a stationary
    # matmul operand: rot = P^T @ q with P[k, m] below.
    p = np.zeros((128, 128), dtype=np.float32)
    for d in range(64):
        p[d + 64, d] = -1.0
        p[d, d + 64] = 1.0
    return p


_NC = None
LAST_RESULTS = None


def kernel(hidden_states, wq, wk, wv, wo):
    global _NC, LAST_RESULTS
    if _NC is None:
        _NC = _build()
    cosT, sinT = _rope_tables()
    ident = np.eye(128, dtype=np.float32)
    onesb = np.ones((128, 128), dtype=ml_dtypes.bfloat16)
    rotP = _rot_perm()
    hs = np.asarray(hidden_states, dtype=np.float32)
    wq = np.asarray(wq, dtype=np.float32)
    wk = np.asarray(wk, dtype=np.float32)
    wv = np.asarray(wv, dtype=np.float32)
    wo = np.asarray(wo, dtype=np.float32)

    in_maps = []
    for core in range(NCORES):
        b, g = divmod(core, G)
        in_maps.append({
            "xTb": np.ascontiguousarray(hs[b].T).reshape(NE, 128, S),
            "wqb": np.ascontiguousarray(
                wq[:, GD * g:GD * (g + 1)]).reshape(NE, 128, GD),
            "wkb": np.ascontiguousarray(
                wk[:, D * g:D * (g + 1)]).reshape(NE, 128, D),
            "wvb": np.ascontiguousarray(
                wv[:, D * g:D * (g + 1)]).reshape(NE, 128, D),
            "wob": np.ascontiguousarray(
                wo[GD * g:GD * (g + 1), :]).reshape(G, 128, E),
            "cosT": cosT,
            "sinT": sinT,
            "rotP": rotP,
            "ident": ident,
            "onesb": onesb,
        })

    res = run_bass_kernel_spmd(_NC, in_maps, list(range(NCORES)))
    LAST_RESULTS = res
    outs = [np.asarray(res.results[i]["out"], dtype=np.float32)
            for i in range(NCORES)]
    full = np.stack([sum(outs[b * G:(b + 1) * G]) for b in range(B)], axis=0)
    return full.astype(np.float32)


# revision 39
# speedup vs baseline: 1.1335x; 1.0613x over previous
"""GQA (16 q-heads / 4 kv-heads, D=128, S=2048, E=2048, B=2) on 8 trn2 cores.

Sharding: core = 4*b + g  (b in {0,1} batch, g in {0..3} kv-head group).
Each core computes its batch's 4 query heads (one kv group) end-to-end and
the host sums the 4 partial o_proj outputs per batch.

v3 (single interleaved program, engines balanced, DMA-consolidated):
  - Blocked DRAM layouts + AP.transpose give one DMA per weight tensor and
    4 DMAs per x position-chunk (~65 DMAs total vs ~250).
  - Phase A per 512-position chunk: K proj+RoPE, Q(h0) proj+RoPE, V proj +
    PE-transpose to natural bf16, Q(h1..h3) proj+RoPE.  RoPE rotate-half is
    a signed-permutation matmul on PE (no SBUF swap DMAs); cos/sin mults
    split DVE/DVE, add on Pool.
  - Attention in four 512-wide query chunks: scoresT per sk-tile in f32r,
    Exp on Act straight from PSUM to bf16 SBUF, bf16 AV matmuls (V-natural
    stationary), softmax denominator via two bf16 chain-adds (DVE + Pool),
    ones-matmul partition reduce, reciprocal, Pool partition_broadcast,
    DVE normalize multiply into f32r ot.
  - o_proj of chunk c interleaved into attention of chunk c+1; stores
    staged through SBUF [128,1024] tiles via DVE/Act copies.
"""

import numpy as np
import ml_dtypes

import concourse.bass as bass
import concourse.bacc as bacc
import concourse.mybir as mybir
import concourse.tile as tile
from concourse.bass_utils import run_bass_kernel_spmd

B, S, E = 2, 2048, 2048
H, HKV, D = 16, 4, 128
G = H // HKV          # 4 query heads per kv group
GD = G * D            # 512 channels per group
NCORES = 8
SCALE = 1.0 / float(np.sqrt(D))
ROPE_BASE = 10000.0

NE = E // 128         # 16 e-blocks (contraction for projections)
NC4 = S // 512        # 4 position chunks of 512 (projection granularity)
NST = S // 128        # 16 sk-tiles of 128
CHS = [512, 1024, 512]          # attention query-chunk widths
COFF = [0, 512, 1536]           # their offsets
CHMAX = 1024

F32 = mybir.dt.float32
F32R = mybir.dt.float32r
BF16 = mybir.dt.bfloat16
AF = mybir.ActivationFunctionType
OP = mybir.AluOpType


def _r(ap):
    return ap.bitcast(F32R)


def _emit(nc, tc, xTb, wqb, wkb, wvb, wob, cosT, sinT, rotP, ident, onesb, out):
    from contextlib import ExitStack
    es = ExitStack()
    with es:
        cpool = es.enter_context(tc.tile_pool(name="const", bufs=1))
        wopool = es.enter_context(tc.tile_pool(name="wo", bufs=2))
        xpool = es.enter_context(tc.tile_pool(name="xs", bufs=16))
        rpool = es.enter_context(tc.tile_pool(name="rope", bufs=2))
        etpool = es.enter_context(tc.tile_pool(name="et", bufs=8))
        bcspool = es.enter_context(tc.tile_pool(name="bcs", bufs=2))
        dnpool = es.enter_context(tc.tile_pool(name="dn", bufs=2))
        rcpool = es.enter_context(tc.tile_pool(name="rc", bufs=2))
        otpool = es.enter_context(tc.tile_pool(name="ot", bufs=6))
        ostgpool = es.enter_context(tc.tile_pool(name="ostg", bufs=4))
        vtpool = es.enter_context(tc.tile_pool(name="vt", bufs=2))
        pssc = es.enter_context(
            tc.tile_pool(name="pssc", bufs=2, space=bass.MemorySpace.PSUM))
        psav = es.enter_context(
            tc.tile_pool(name="psav", bufs=1, space=bass.MemorySpace.PSUM))
        psmx = es.enter_context(
            tc.tile_pool(name="psmx", bufs=2, space=bass.MemorySpace.PSUM))

        # ---- small constants ----
        id_sb = cpool.tile([128, 128], F32, tag="id")
        ones_sb = cpool.tile([128, 128], BF16, tag="ones")
        rp_sb = cpool.tile([128, 128], BF16, tag="rp")
        nc.sync.dma_start(out=id_sb[:], in_=ident.ap())
        nc.sync.dma_start(out=ones_sb[:], in_=onesb.ap())
        nc.sync.dma_start(out=rp_sb[:], in_=rotP.ap())
        cos_sb = cpool.tile([D, S], BF16, tag="cos")
        sin_sb = cpool.tile([D, S], BF16, tag="sin")
        nc.sync.dma_start(out=cos_sb[:], in_=cosT.ap())
        nc.sync.dma_start(out=sin_sb[:], in_=sinT.ap())

        # ---- consolidated weight loads; wq/wv DMAs are issued after the
        # first x chunk so the critical path to the first K matmul is short
        wk_t = cpool.tile([128, NE, D], BF16, tag="wkt")
        nc.sync.dma_start(out=wk_t[:],
                          in_=wkb.ap().transpose([1, 0, 2]))
        wq_t = cpool.tile([128, NE, GD], BF16, tag="wqt")
        wv_t = cpool.tile([128, NE, D], BF16, tag="wvt")

        # ---- persistent activations (bf16: same PE rate, half SBUF) ----
        kt = cpool.tile([D, S], BF16, tag="kt")
        qt = [cpool.tile([D, S], BF16, tag=f"qt{h}", name=f"qt{h}")
              for h in range(G)]
        vn = cpool.tile([128, NST, D], BF16, tag="vn")

        def rope(dst, ps, sl, in_attn=False):
            # rotate_half as a signed-permutation matmul; then
            # dst = q*cos + rot(q)*sin.  During phase A the qraw copy runs on
            # the idle Act engine and rot borrows an idle scores-pool PSUM
            # slot; inside attention windows both would collide with exp /
            # scores, so qraw moves to DVE and rot stays in psmx.
            qraw = rpool.tile([128, 512], BF16, tag="qraw")
            tmc = rpool.tile([128, 512], F32, tag="tmc")
            t2 = rpool.tile([128, 512], F32, tag="t2")
            if in_attn:
                nc.vector.tensor_copy(qraw[:], ps[:])
                rot = psmx.tile([128, 512], F32, tag="mx", name="rot")
            else:
                nc.scalar.copy(qraw[:], ps[:])
                rot = pssc.tile([128, 512], F32, tag="sc", name="rot")
            nc.tensor.matmul(rot[:], rp_sb[:], qraw[:], start=True, stop=True)
            nc.vector.tensor_tensor(tmc[:], qraw[:], cos_sb[:, sl], OP.mult)
            nc.vector.tensor_tensor(t2[:], rot[:], sin_sb[:, sl], OP.mult)
            nc.gpsimd.tensor_tensor(dst, tmc[:], t2[:], OP.add)

        def load_x(c4):
            sl = slice(c4 * 512, (c4 + 1) * 512)
            xsl = [xpool.tile([128, 2, 512], BF16, tag="xs",
                              name=f"xs{c4}_{jj}") for jj in range(8)]
            for jj in range(8):
                nc.sync.dma_start(
                    out=xsl[jj][:],
                    in_=xTb.ap()[jj * 2:(jj + 1) * 2, :, sl]
                        .transpose([1, 0, 2]))
            return xsl

        def proj(wt, cslice, xsl):
            ps = psmx.tile([128, 512], F32, tag="mx", name="ps")
            for j in range(NE):
                nc.tensor.matmul(ps[:], wt[:, j, cslice],
                                 xsl[j // 2][:, j % 2, :],
                                 start=(j == 0), stop=(j == NE - 1))
            return ps

        def qproj(h, c4, xsl):
            sl = slice(c4 * 512, (c4 + 1) * 512)
            ps = proj(wq_t, slice(h * D, (h + 1) * D), xsl)
            rope(qt[h][:, sl], ps, sl)

        # ============ phase A: K/V for all chunks, Q for chunk 0 ============
        # Remaining Q projections are interleaved into the act-gated
        # attention windows below (x slices reloaded through the same pool).
        for c4 in range(NC4):
            sl = slice(c4 * 512, (c4 + 1) * 512)
            xsl = load_x(c4)
            if c4 == 0:
                for jj in range(4):
                    nc.sync.dma_start(
                        out=wq_t[:, jj * 4:(jj + 1) * 4, :],
                        in_=wqb.ap()[jj * 4:(jj + 1) * 4, :, :]
                            .transpose([1, 0, 2]))
                nc.sync.dma_start(
                    out=wv_t[:],
                    in_=wvb.ap().transpose([1, 0, 2]))
            # K
            ps = proj(wk_t, slice(0, D), xsl)
            rope(kt[:, sl], ps, sl)
            # V -> natural bf16 via PE transpose
            ps = proj(wv_t, slice(0, D), xsl)
            vt = vtpool.tile([D, 512], F32, tag="vt")
            nc.scalar.copy(vt[:], ps[:])
            for tt in range(4):
                trp = psmx.tile([128, 128], F32, tag="mx", name="trp")
                nc.tensor.transpose(trp[:], vt[:, tt * 128:(tt + 1) * 128],
                                    id_sb[:])
                nc.vector.tensor_copy(vn[:, c4 * 4 + tt, :], trp[:])
            if c4 == 0:
                for h in range(G):
                    qproj(h, 0, xsl)

        # ================= phases B+C: attention + o_proj =================
        # Deferred PE work (remaining Q projections, previous-chunk o_proj)
        # is queued as ~1-matmul units and drained a couple per t-iteration,
        # so the Act engine never sees a multi-microsecond scores gap.
        import collections
        fill = collections.deque()

        def drain(n):
            while n > 0 and fill:
                fill.popleft()()
                n -= 1

        def enqueue_qproj(h, c4, xsl):
            sl = slice(c4 * 512, (c4 + 1) * 512)
            state = {}

            def mk(j):
                def unit():
                    if j == 0:
                        state['ps'] = psmx.tile([128, 512], F32, tag="mx",
                                                name="ps")
                    ps = state['ps']
                    for jj in (2 * j, 2 * j + 1):
                        nc.tensor.matmul(ps[:], wq_t[:, jj, h * D:(h + 1) * D],
                                         xsl[jj // 2][:, jj % 2, :],
                                         start=(jj == 0), stop=(jj == NE - 1))
                return unit
            for j in range(NE // 2):
                fill.append(mk(j))
            fill.append(lambda: rope(qt[h][:, sl], state['ps'], sl,
                                     in_attn=True))

        def attn_head(off, chw, h):
            """Scores/exp/AV/denominator/normalize for a chw-wide query chunk
            at offset off, head h.  Returns the normalized [D, chw] f32r ot."""
            nhf = chw // 512
            dn = dnpool.tile([128, chw], BF16, tag="dn")
            av = psav.tile([D, chw], F32, tag="av")
            et_first = None
            for t in range(NST):
                sc = pssc.tile([128, chw], F32, tag="sc")
                for hf in range(nhf):
                    qsl = slice(off + hf * 512, off + (hf + 1) * 512)
                    nc.tensor.matmul(sc[:, hf * 512:(hf + 1) * 512],
                                     kt[:, t * 128:(t + 1) * 128],
                                     qt[h][:, qsl], start=True, stop=True)
                et = etpool.tile([128, chw], BF16, tag="et")
                nc.scalar.activation(et[:], sc[:], AF.Exp, scale=SCALE)
                for hf in range(nhf):
                    nc.tensor.matmul(av[:, hf * 512:(hf + 1) * 512],
                                     vn[:, t, :],
                                     et[:, hf * 512:(hf + 1) * 512],
                                     start=(t == 0), stop=(t == NST - 1))
                # denominator: single DVE chain, always caught up with exp
                if t == 0:
                    et_first = et
                elif t == 1:
                    nc.vector.tensor_tensor(dn[:], et_first[:], et[:], OP.add)
                else:
                    nc.vector.tensor_tensor(dn[:], dn[:], et[:], OP.add)
                if t >= 2:
                    drain(2)
            rc = rcpool.tile([1, chw], BF16, tag="rc")
            for hf in range(nhf):
                sm = psmx.tile([1, 512], F32, tag="mx", name="sm")
                nc.tensor.matmul(sm[:], ones_sb[:, 0:1],
                                 dn[:, hf * 512:(hf + 1) * 512],
                                 start=True, stop=True)
                with nc.allow_low_precision(reason="bf16 softmax denom recip"):
                    nc.vector.reciprocal(rc[:, hf * 512:(hf + 1) * 512], sm[:])
            bcs = bcspool.tile([128, chw], BF16, tag="bcs")
            nc.gpsimd.partition_broadcast(bcs[:], rc[:])
            ot = otpool.tile([D, chw], BF16, tag="ot")
            nc.vector.tensor_tensor(ot[:], av[:], bcs[:], OP.mult)
            return ot

        def enqueue_oproj_pair(off, chw, eo0, ots, ci, copy_eng):
            """Two adjacent eo column-groups (1024 cols of E) of o_proj for
            the chunk at offset off: one [128,1024] staging tile per s-tile,
            one store each.  Each (st, k) group is one filler unit."""
            wots = []
            for eo in (eo0, eo0 + 1):
                wot = wopool.tile([128, G, 512], BF16, tag="wo",
                                  name=f"wo{ci}_{eo}")
                nc.sync.dma_start(
                    out=wot[:],
                    in_=wob.ap()[:, :, eo * 512:(eo + 1) * 512]
                        .transpose([1, 0, 2]))
                wots.append(wot)
            for st in range(chw // 128):
                state = {}

                def mk(st, k):
                    def unit():
                        if k == 0:
                            state['ostg'] = ostgpool.tile(
                                [128, 1024], F32, tag="ostg",
                                name=f"ostg{ci}_{eo0}_{st}")
                        ostg = state['ostg']
                        op = psmx.tile([128, 512], F32, tag="mx", name="op")
                        for h in range(G):
                            nc.tensor.matmul(
                                op[:], ots[h][:, st * 128:(st + 1) * 128],
                                wots[k][:, h, :],
                                start=(h == 0), stop=(h == G - 1))
                        dst = ostg[:, k * 512:(k + 1) * 512]
                        if copy_eng == 'act':
                            nc.scalar.copy(dst, op[:])
                        else:
                            nc.vector.tensor_copy(dst, op[:])
                        if k == 1:
                            nc.sync.dma_start(
                                out=out.ap()[off + st * 128:
                                             off + (st + 1) * 128,
                                             eo0 * 512:(eo0 + 2) * 512],
                                in_=ostg[:])
                    return unit
                fill.append(mk(st, 0))
                fill.append(mk(st, 1))

        # B0 (512-wide): deferred Q projections for position chunks 1 and 2
        # drain into the PE slack of the attention loop.  x slices are
        # re-loaded one stage ahead so filler matmuls never wait on DMA.
        xs1 = load_x(1)
        ots0, ots1, ots2 = [], [], []
        xs2 = None
        for h in range(G):
            if h == 0:
                for hq in range(G):
                    enqueue_qproj(hq, 1, xs1)
            elif h == 1:
                xs2 = load_x(2)
            elif h == 2:
                for hq in range(G):
                    enqueue_qproj(hq, 2, xs2)
            ots0.append(attn_head(COFF[0], CHS[0], h))
            drain(8)
        drain(len(fill))
        # B1 (1024-wide): o_proj of chunk 0 + Q projections for chunk 3
        xs3 = load_x(3)
        for h in range(G):
            if h == 0:
                enqueue_oproj_pair(COFF[0], CHS[0], 0, ots0, 0, 'dve')
                for hq in range(G):
                    enqueue_qproj(hq, 3, xs3)
                enqueue_oproj_pair(COFF[0], CHS[0], 2, ots0, 0, 'dve')
            ots1.append(attn_head(COFF[1], CHS[1], h))
            drain(8)
        drain(len(fill))
        # B2 (512-wide): o_proj of chunk 1
        for h in range(G):
            if h == 0:
                enqueue_oproj_pair(COFF[1], CHS[1], 0, ots1, 1, 'dve')
                enqueue_oproj_pair(COFF[1], CHS[1], 2, ots1, 1, 'dve')
            ots2.append(attn_head(COFF[2], CHS[2], h))
            drain(8)
        drain(len(fill))
        # tail: o_proj of chunk 2, staging copies on the now-idle Act engine
        enqueue_oproj_pair(COFF[2], CHS[2], 0, ots2, 2, 'act')
        enqueue_oproj_pair(COFF[2], CHS[2], 2, ots2, 2, 'act')
        drain(len(fill))


def _build():
    nc = bacc.Bacc("TRN2", target_bir_lowering=False, debug=False,
                   num_devices=NCORES)
    xTb = nc.dram_tensor("xTb", [NE, 128, S], BF16, kind="ExternalInput")
    wqb = nc.dram_tensor("wqb", [NE, 128, GD], BF16, kind="ExternalInput")
    wkb = nc.dram_tensor("wkb", [NE, 128, D], BF16, kind="ExternalInput")
    wvb = nc.dram_tensor("wvb", [NE, 128, D], BF16, kind="ExternalInput")
    wob = nc.dram_tensor("wob", [G, 128, E], BF16, kind="ExternalInput")
    cosT = nc.dram_tensor("cosT", [D, S], BF16, kind="ExternalInput")
    sinT = nc.dram_tensor("sinT", [D, S], BF16, kind="ExternalInput")
    rotP = nc.dram_tensor("rotP", [128, 128], BF16, kind="ExternalInput")
    ident = nc.dram_tensor("ident", [128, 128], F32, kind="ExternalInput")
    onesb = nc.dram_tensor("onesb", [128, 128], BF16, kind="ExternalInput")
    out = nc.dram_tensor("out", [S, E], F32, kind="ExternalOutput")
    with tile.TileContext(nc) as tc:
        _emit(nc, tc, xTb, wqb, wkb, wvb, wob, cosT, sinT, rotP, ident,
              onesb, out)
    nc.compile()
    return nc


def _rope_tables():
    inv = 1.0 / (ROPE_BASE ** (np.arange(0, D, 2, dtype=np.float64) / D))
    t = np.arange(S, dtype=np.float64)
    freqs = t[:, None] * inv[None, :]                    # [S, D/2]
    emb = np.concatenate([freqs, freqs], axis=-1)        # [S, D]
    cosT = np.cos(emb).T.astype(ml_dtypes.bfloat16)      # [D, S]
    sinT = np.sin(emb).T.astype(ml_dtypes.bfloat16)
    return np.ascontiguousarray(cosT), np.ascontiguousarray(sinT)


def _rot_perm():
    # rot(q)[d] = -q[d+64] for d<64, +q[d-64] for d>=64, as a stationary
    # matmul operand: rot = P^T @ q with P[k, m] below.
    p = np.zeros((128, 128), dtype=ml_dtypes.bfloat16)
    for d in range(64):
        p[d + 64, d] = -1.0
        p[d, d + 64] = 1.0
    return p


_NC = None
LAST_RESULTS = None


def kernel(hidden_states, wq, wk, wv, wo):
    global _NC, LAST_RESULTS
    if _NC is None:
        _NC = _build()
    cosT, sinT = _rope_tables()
    ident = np.eye(128, dtype=np.float32)
    onesb = np.ones((128, 128), dtype=ml_dtypes.bfloat16)
    rotP = _rot_perm()
    bf = ml_dtypes.bfloat16
    hs = np.asarray(hidden_states, dtype=np.float32)
    wq = np.asarray(wq, dtype=np.float32).astype(bf)
    wk = np.asarray(wk, dtype=np.float32).astype(bf)
    wv = np.asarray(wv, dtype=np.float32).astype(bf)
    wo = np.asarray(wo, dtype=np.float32).astype(bf)

    in_maps = []
    for core in range(NCORES):
        b, g = divmod(core, G)
        in_maps.append({
            "xTb": np.ascontiguousarray(hs[b].T).astype(bf).reshape(NE, 128, S),
            "wqb": np.ascontiguousarray(
                wq[:, GD * g:GD * (g + 1)]).reshape(NE, 128, GD),
            "wkb": np.ascontiguousarray(
                wk[:, D * g:D * (g + 1)]).reshape(NE, 128, D),
            "wvb": np.ascontiguousarray(
                wv[:, D * g:D * (g + 1)]).reshape(NE, 128, D),
            "wob": np.ascontiguousarray(
                wo[GD * g:GD * (g + 1), :]).reshape(G, 128, E),
            "cosT": cosT,
            "sinT": sinT,
            "rotP": rotP,
            "ident": ident,
            "onesb": onesb,
        })

    res = run_bass_kernel_spmd(_NC, in_maps, list(range(NCORES)))
    LAST_RESULTS = res
    outs = [np.asarray(res.results[i]["out"], dtype=np.float32)
            for i in range(NCORES)]
    full = np.stack([sum(outs[b * G:(b + 1) * G]) for b in range(B)], axis=0)
    return full.astype(np.float32)


# revision 41
# speedup vs baseline: 1.2435x; 1.0971x over previous
"""GQA (16 q-heads / 4 kv-heads, D=128, S=2048, E=2048, B=2) on 8 trn2 cores.

Sharding: core = 4*b + g  (b in {0,1} batch, g in {0..3} kv-head group).
Each core computes its batch's 4 query heads (one kv group) end-to-end and
the host sums the 4 partial o_proj outputs per batch.

v3 (single interleaved program, engines balanced, DMA-consolidated):
  - Blocked DRAM layouts + AP.transpose give one DMA per weight tensor and
    4 DMAs per x position-chunk (~65 DMAs total vs ~250).
  - Phase A per 512-position chunk: K proj+RoPE, Q(h0) proj+RoPE, V proj +
    PE-transpose to natural bf16, Q(h1..h3) proj+RoPE.  RoPE rotate-half is
    a signed-permutation matmul on PE (no SBUF swap DMAs); cos/sin mults
    split DVE/DVE, add on Pool.
  - Attention in four 512-wide query chunks: scoresT per sk-tile in f32r,
    Exp on Act straight from PSUM to bf16 SBUF, bf16 AV matmuls (V-natural
    stationary), softmax denominator via two bf16 chain-adds (DVE + Pool),
    ones-matmul partition reduce, reciprocal, Pool partition_broadcast,
    DVE normalize multiply into f32r ot.
  - o_proj of chunk c interleaved into attention of chunk c+1; stores
    staged through SBUF [128,1024] tiles via DVE/Act copies.
"""

import numpy as np
import ml_dtypes

import concourse.bass as bass
import concourse.bacc as bacc
import concourse.mybir as mybir
import concourse.tile as tile
from concourse.bass_utils import run_bass_kernel_spmd

B, S, E = 2, 2048, 2048
H, HKV, D = 16, 4, 128
G = H // HKV          # 4 query heads per kv group
GD = G * D            # 512 channels per group
NCORES = 8
SCALE = 1.0 / float(np.sqrt(D))
ROPE_BASE = 10000.0
AX = 16.0             # fp8 plane scale for x
AW = 64.0             # fp8 plane scale for wq/wk/wv
PSC = AX * AW         # q/k/v come out scaled by PSC
SCALE_EFF = SCALE / (PSC * PSC)   # folds the q*k scale into exp

NE = E // 128         # 16 e-blocks (contraction for projections)
NC4 = S // 512        # 4 position chunks of 512 (projection granularity)
NST = S // 128        # 16 sk-tiles of 128
CHS = [512, 1024, 512]          # attention query-chunk widths
COFF = [0, 512, 1536]           # their offsets
CHMAX = 1024

F32 = mybir.dt.float32
F32R = mybir.dt.float32r
BF16 = mybir.dt.bfloat16
FP8 = mybir.dt.float8e4
DR = mybir.MatmulPerfMode.DoubleRow
AF = mybir.ActivationFunctionType
OP = mybir.AluOpType


def _r(ap):
    return ap.bitcast(F32R)


def _emit(nc, tc, xh, xl, wqh, wql, wkh, wkl, wvh, wvl, wob, cosT, sinT, rotP, ident, onesb, out):
    from contextlib import ExitStack
    es = ExitStack()
    with es:
        cpool = es.enter_context(tc.tile_pool(name="const", bufs=1))
        wopool = es.enter_context(tc.tile_pool(name="wo", bufs=2))
        xpool = es.enter_context(tc.tile_pool(name="xs", bufs=16))
        rpool = es.enter_context(tc.tile_pool(name="rope", bufs=2))
        etpool = es.enter_context(tc.tile_pool(name="et", bufs=8))
        bcspool = es.enter_context(tc.tile_pool(name="bcs", bufs=2))
        dnpool = es.enter_context(tc.tile_pool(name="dn", bufs=2))
        rcpool = es.enter_context(tc.tile_pool(name="rc", bufs=2))
        otpool = es.enter_context(tc.tile_pool(name="ot", bufs=6))
        ostgpool = es.enter_context(tc.tile_pool(name="ostg", bufs=4))
        vtpool = es.enter_context(tc.tile_pool(name="vt", bufs=2))
        pssc = es.enter_context(
            tc.tile_pool(name="pssc", bufs=2, space=bass.MemorySpace.PSUM))
        psav = es.enter_context(
            tc.tile_pool(name="psav", bufs=1, space=bass.MemorySpace.PSUM))
        psmx = es.enter_context(
            tc.tile_pool(name="psmx", bufs=2, space=bass.MemorySpace.PSUM))

        # ---- small constants ----
        id_sb = cpool.tile([128, 128], F32, tag="id")
        ones_sb = cpool.tile([128, 128], BF16, tag="ones")
        rp_sb = cpool.tile([128, 128], BF16, tag="rp")
        nc.sync.dma_start(out=id_sb[:], in_=ident.ap())
        nc.sync.dma_start(out=ones_sb[:], in_=onesb.ap())
        nc.sync.dma_start(out=rp_sb[:], in_=rotP.ap())
        cos_sb = cpool.tile([D, S], BF16, tag="cos")
        sin_sb = cpool.tile([D, S], BF16, tag="sin")
        nc.sync.dma_start(out=cos_sb[:], in_=cosT.ap())
        nc.sync.dma_start(out=sin_sb[:], in_=sinT.ap())

        # ---- consolidated weight loads (fp8 hi/lo double-quant planes);
        # wq/wv DMAs are issued after the first x chunk so the critical path
        # to the first K matmul is short
        wk_t = [cpool.tile([128, NE, D], FP8, tag=f"wkt{i}", name=f"wkt{i}")
                for i in range(2)]
        for i, t in enumerate((wkh, wkl)):
            nc.sync.dma_start(out=wk_t[i][:], in_=t.ap().transpose([1, 0, 2]))
        wq_t = [cpool.tile([128, NE, GD], FP8, tag=f"wqt{i}", name=f"wqt{i}")
                for i in range(2)]
        wv_t = [cpool.tile([128, NE, D], FP8, tag=f"wvt{i}", name=f"wvt{i}")
                for i in range(2)]

        # ---- persistent activations (bf16: same PE rate, half SBUF) ----
        kt = cpool.tile([D, S], BF16, tag="kt")
        qt = [cpool.tile([D, S], BF16, tag=f"qt{h}", name=f"qt{h}")
              for h in range(G)]
        vn = cpool.tile([128, NST, D], BF16, tag="vn")

        def rope(dst, ps, sl, in_attn=False):
            # rotate_half as a signed-permutation matmul; then
            # dst = q*cos + rot(q)*sin.  During phase A the qraw copy runs on
            # the idle Act engine and rot borrows an idle scores-pool PSUM
            # slot; inside attention windows both would collide with exp /
            # scores, so qraw moves to DVE and rot stays in psmx.
            qraw = rpool.tile([128, 512], BF16, tag="qraw")
            tmc = rpool.tile([128, 512], F32, tag="tmc")
            t2 = rpool.tile([128, 512], F32, tag="t2")
            if in_attn:
                nc.vector.tensor_copy(qraw[:], ps[:])
                rot = psmx.tile([128, 512], F32, tag="mx", name="rot")
            else:
                nc.scalar.copy(qraw[:], ps[:])
                rot = pssc.tile([128, 512], F32, tag="sc", name="rot")
            nc.tensor.matmul(rot[:], rp_sb[:], qraw[:], start=True, stop=True)
            nc.vector.tensor_tensor(tmc[:], qraw[:], cos_sb[:, sl], OP.mult)
            nc.vector.tensor_tensor(t2[:], rot[:], sin_sb[:, sl], OP.mult)
            nc.gpsimd.tensor_tensor(dst, tmc[:], t2[:], OP.add)

        def load_x(c4):
            sl = slice(c4 * 512, (c4 + 1) * 512)
            xsl = []
            for i, t in enumerate((xh, xl)):
                tiles = [xpool.tile([128, 4, 512], FP8, tag="xs",
                                    name=f"xs{c4}_{i}_{jj}")
                         for jj in range(4)]
                for jj in range(4):
                    nc.sync.dma_start(
                        out=tiles[jj][:],
                        in_=t.ap()[jj * 4:(jj + 1) * 4, :, sl]
                            .transpose([1, 0, 2]))
                xsl.append(tiles)
            return xsl

        PLANES = ((0, 0), (0, 1), (1, 0))   # (w_plane, x_plane): HH, HL, LH

        def proj_mms(ps, wt, cslice, xsl):
            mms = []
            for i, (wi, xi) in enumerate(PLANES):
                for p in range(NE // 2):
                    mms.append((wt[wi][:, 2 * p:2 * p + 2, cslice],
                                xsl[xi][p // 2][:, 2 * (p % 2):2 * (p % 2) + 2, :]))
            return mms

        def proj(wt, cslice, xsl):
            ps = psmx.tile([128, 512], F32, tag="mx", name="ps")
            mms = proj_mms(ps, wt, cslice, xsl)
            for i, (wa, xa) in enumerate(mms):
                nc.tensor.matmul(ps[:], wa, xa, perf_mode=DR,
                                 start=(i == 0), stop=(i == len(mms) - 1))
            return ps

        def qproj(h, c4, xsl):
            sl = slice(c4 * 512, (c4 + 1) * 512)
            ps = proj(wq_t, slice(h * D, (h + 1) * D), xsl)
            rope(qt[h][:, sl], ps, sl)

        # ============ phase A: K/V for all chunks, Q for chunk 0 ============
        # Remaining Q projections are interleaved into the act-gated
        # attention windows below (x slices reloaded through the same pool).
        for c4 in range(NC4):
            sl = slice(c4 * 512, (c4 + 1) * 512)
            xsl = load_x(c4)
            if c4 == 0:
                for i, t in enumerate((wqh, wql)):
                    for jj in range(2):
                        nc.sync.dma_start(
                            out=wq_t[i][:, jj * 8:(jj + 1) * 8, :],
                            in_=t.ap()[jj * 8:(jj + 1) * 8, :, :]
                                .transpose([1, 0, 2]))
                for i, t in enumerate((wvh, wvl)):
                    nc.sync.dma_start(out=wv_t[i][:],
                                      in_=t.ap().transpose([1, 0, 2]))
            # K
            ps = proj(wk_t, slice(0, D), xsl)
            rope(kt[:, sl], ps, sl)
            # V -> natural bf16 via PE transpose
            ps = proj(wv_t, slice(0, D), xsl)
            vt = vtpool.tile([D, 512], F32, tag="vt")
            nc.scalar.copy(vt[:], ps[:])
            for tt in range(4):
                trp = psmx.tile([128, 128], F32, tag="mx", name="trp")
                nc.tensor.transpose(trp[:], vt[:, tt * 128:(tt + 1) * 128],
                                    id_sb[:])
                nc.vector.tensor_copy(vn[:, c4 * 4 + tt, :], trp[:])
            if c4 == 0:
                for h in range(G):
                    qproj(h, 0, xsl)

        # ================= phases B+C: attention + o_proj =================
        # Deferred PE work (remaining Q projections, previous-chunk o_proj)
        # is queued as ~1-matmul units and drained a couple per t-iteration,
        # so the Act engine never sees a multi-microsecond scores gap.
        import collections
        fill = collections.deque()

        def drain(n):
            while n > 0 and fill:
                fill.popleft()()
                n -= 1

        def enqueue_qproj(h, c4, xsl):
            sl = slice(c4 * 512, (c4 + 1) * 512)
            state = {}
            cslice = slice(h * D, (h + 1) * D)
            nmm = 3 * (NE // 2)

            def mk(j):
                def unit():
                    if j == 0:
                        state['ps'] = psmx.tile([128, 512], F32, tag="mx",
                                                name="ps")
                        state['mms'] = proj_mms(state['ps'], wq_t, cslice, xsl)
                    ps = state['ps']
                    for jj in (2 * j, 2 * j + 1):
                        wa, xa = state['mms'][jj]
                        nc.tensor.matmul(ps[:], wa, xa, perf_mode=DR,
                                         start=(jj == 0), stop=(jj == nmm - 1))
                return unit
            for j in range(nmm // 2):
                fill.append(mk(j))
            fill.append(lambda: rope(qt[h][:, sl], state['ps'], sl,
                                     in_attn=True))

        def attn_head(off, chw, h):
            """Scores/exp/AV/denominator/normalize for a chw-wide query chunk
            at offset off, head h.  Returns the normalized [D, chw] f32r ot."""
            nhf = chw // 512
            dn = dnpool.tile([128, chw], BF16, tag="dn")
            av = psav.tile([D, chw], F32, tag="av")
            et_first = None
            for t in range(NST):
                sc = pssc.tile([128, chw], F32, tag="sc")
                for hf in range(nhf):
                    qsl = slice(off + hf * 512, off + (hf + 1) * 512)
                    nc.tensor.matmul(sc[:, hf * 512:(hf + 1) * 512],
                                     kt[:, t * 128:(t + 1) * 128],
                                     qt[h][:, qsl], start=True, stop=True)
                et = etpool.tile([128, chw], BF16, tag="et")
                nc.scalar.activation(et[:], sc[:], AF.Exp, scale=SCALE_EFF)
                for hf in range(nhf):
                    nc.tensor.matmul(av[:, hf * 512:(hf + 1) * 512],
                                     vn[:, t, :],
                                     et[:, hf * 512:(hf + 1) * 512],
                                     start=(t == 0), stop=(t == NST - 1))
                # denominator: single DVE chain, always caught up with exp
                if t == 0:
                    et_first = et
                elif t == 1:
                    nc.vector.tensor_tensor(dn[:], et_first[:], et[:], OP.add)
                else:
                    nc.vector.tensor_tensor(dn[:], dn[:], et[:], OP.add)
                if t >= 2:
                    drain(2)
            rc = rcpool.tile([1, chw], BF16, tag="rc")
            for hf in range(nhf):
                sm = psmx.tile([1, 512], F32, tag="mx", name="sm")
                nc.tensor.matmul(sm[:], ones_sb[:, 0:1],
                                 dn[:, hf * 512:(hf + 1) * 512],
                                 start=True, stop=True)
                with nc.allow_low_precision(reason="bf16 softmax denom recip"):
                    nc.vector.reciprocal(rc[:, hf * 512:(hf + 1) * 512], sm[:])
            bcs = bcspool.tile([128, chw], BF16, tag="bcs")
            nc.gpsimd.partition_broadcast(bcs[:], rc[:])
            ot = otpool.tile([D, chw], BF16, tag="ot")
            nc.vector.tensor_tensor(ot[:], av[:], bcs[:], OP.mult)
            return ot

        def enqueue_oproj_pair(off, chw, eo0, ots, ci, copy_eng):
            """Two adjacent eo column-groups (1024 cols of E) of o_proj for
            the chunk at offset off: one [128,1024] staging tile per s-tile,
            one store each.  Each (st, k) group is one filler unit."""
            wots = []
            for eo in (eo0, eo0 + 1):
                wot = wopool.tile([128, G, 512], BF16, tag="wo",
                                  name=f"wo{ci}_{eo}")
                nc.sync.dma_start(
                    out=wot[:],
                    in_=wob.ap()[:, :, eo * 512:(eo + 1) * 512]
                        .transpose([1, 0, 2]))
                wots.append(wot)
            for st in range(chw // 128):
                state = {}

                def mk(st, k):
                    def unit():
                        if k == 0:
                            state['ostg'] = ostgpool.tile(
                                [128, 1024], F32, tag="ostg",
                                name=f"ostg{ci}_{eo0}_{st}")
                        ostg = state['ostg']
                        op = psmx.tile([128, 512], F32, tag="mx", name="op")
                        for h in range(G):
                            nc.tensor.matmul(
                                op[:], ots[h][:, st * 128:(st + 1) * 128],
                                wots[k][:, h, :],
                                start=(h == 0), stop=(h == G - 1))
                        dst = ostg[:, k * 512:(k + 1) * 512]
                        if copy_eng == 'act':
                            nc.scalar.copy(dst, op[:])
                        else:
                            nc.vector.tensor_copy(dst, op[:])
                        if k == 1:
                            nc.sync.dma_start(
                                out=out.ap()[off + st * 128:
                                             off + (st + 1) * 128,
                                             eo0 * 512:(eo0 + 2) * 512],
                                in_=ostg[:])
                    return unit
                fill.append(mk(st, 0))
                fill.append(mk(st, 1))

        # B0 (512-wide): deferred Q projections for position chunks 1 and 2
        # drain into the PE slack of the attention loop.  x slices are
        # re-loaded one stage ahead so filler matmuls never wait on DMA.
        xs1 = load_x(1)
        ots0, ots1, ots2 = [], [], []
        xs2 = None
        for h in range(G):
            if h == 0:
                for hq in range(G):
                    enqueue_qproj(hq, 1, xs1)
            elif h == 1:
                xs2 = load_x(2)
            elif h == 2:
                for hq in range(G):
                    enqueue_qproj(hq, 2, xs2)
            ots0.append(attn_head(COFF[0], CHS[0], h))
            drain(8)
        drain(len(fill))
        # B1 (1024-wide): o_proj of chunk 0 + Q projections for chunk 3
        xs3 = load_x(3)
        for h in range(G):
            if h == 0:
                enqueue_oproj_pair(COFF[0], CHS[0], 0, ots0, 0, 'dve')
                for hq in range(G):
                    enqueue_qproj(hq, 3, xs3)
                enqueue_oproj_pair(COFF[0], CHS[0], 2, ots0, 0, 'dve')
            ots1.append(attn_head(COFF[1], CHS[1], h))
            drain(8)
        drain(len(fill))
        # B2 (512-wide): o_proj of chunk 1, spread across all four heads
        for h in range(G):
            if h == 0:
                enqueue_oproj_pair(COFF[1], CHS[1], 0, ots1, 1, 'dve')
            elif h == 2:
                enqueue_oproj_pair(COFF[1], CHS[1], 2, ots1, 1, 'dve')
            ots2.append(attn_head(COFF[2], CHS[2], h))
            drain(8)
        drain(len(fill))
        # tail: o_proj of chunk 2, staging copies on the now-idle Act engine
        enqueue_oproj_pair(COFF[2], CHS[2], 0, ots2, 2, 'act')
        enqueue_oproj_pair(COFF[2], CHS[2], 2, ots2, 2, 'act')
        drain(len(fill))


def _build():
    nc = bacc.Bacc("TRN2", target_bir_lowering=False, debug=False,
                   num_devices=NCORES)
    xh = nc.dram_tensor("xh", [NE, 128, S], FP8, kind="ExternalInput")
    xl = nc.dram_tensor("xl", [NE, 128, S], FP8, kind="ExternalInput")
    wqh = nc.dram_tensor("wqh", [NE, 128, GD], FP8, kind="ExternalInput")
    wql = nc.dram_tensor("wql", [NE, 128, GD], FP8, kind="ExternalInput")
    wkh = nc.dram_tensor("wkh", [NE, 128, D], FP8, kind="ExternalInput")
    wkl = nc.dram_tensor("wkl", [NE, 128, D], FP8, kind="ExternalInput")
    wvh = nc.dram_tensor("wvh", [NE, 128, D], FP8, kind="ExternalInput")
    wvl = nc.dram_tensor("wvl", [NE, 128, D], FP8, kind="ExternalInput")
    wob = nc.dram_tensor("wob", [G, 128, E], BF16, kind="ExternalInput")
    cosT = nc.dram_tensor("cosT", [D, S], BF16, kind="ExternalInput")
    sinT = nc.dram_tensor("sinT", [D, S], BF16, kind="ExternalInput")
    rotP = nc.dram_tensor("rotP", [128, 128], BF16, kind="ExternalInput")
    ident = nc.dram_tensor("ident", [128, 128], F32, kind="ExternalInput")
    onesb = nc.dram_tensor("onesb", [128, 128], BF16, kind="ExternalInput")
    out = nc.dram_tensor("out", [S, E], F32, kind="ExternalOutput")
    with tile.TileContext(nc) as tc:
        _emit(nc, tc, xh, xl, wqh, wql, wkh, wkl, wvh, wvl, wob, cosT, sinT,
              rotP, ident, onesb, out)
    nc.compile()
    return nc


def _rope_tables():
    inv = 1.0 / (ROPE_BASE ** (np.arange(0, D, 2, dtype=np.float64) / D))
    t = np.arange(S, dtype=np.float64)
    freqs = t[:, None] * inv[None, :]                    # [S, D/2]
    emb = np.concatenate([freqs, freqs], axis=-1)        # [S, D]
    cosT = np.cos(emb).T.astype(ml_dtypes.bfloat16)      # [D, S]
    sinT = np.sin(emb).T.astype(ml_dtypes.bfloat16)
    return np.ascontiguousarray(cosT), np.ascontiguousarray(sinT)


def _rot_perm():
    # rot(q)[d] = -q[d+64] for d<64, +q[d-64] for d>=64, as a stationary
    # matmul operand: rot = P^T @ q with P[k, m] below.
    p = np.zeros((128, 128), dtype=ml_dtypes.bfloat16)
    for d in range(64):
        p[d + 64, d] = -1.0
        p[d, d + 64] = 1.0
    return p


_NC = None
LAST_RESULTS = None


def kernel(hidden_states, wq, wk, wv, wo):
    global _NC, LAST_RESULTS
    if _NC is None:
        _NC = _build()
    cosT, sinT = _rope_tables()
    ident = np.eye(128, dtype=np.float32)
    onesb = np.ones((128, 128), dtype=ml_dtypes.bfloat16)
    rotP = _rot_perm()
    bf = ml_dtypes.bfloat16
    f8 = ml_dtypes.float8_e4m3

    def planes(a, scale):
        hi = (scale * a).astype(f8)
        lo = (scale * a - hi.astype(np.float32)).astype(f8)
        return hi, lo

    hs = np.asarray(hidden_states, dtype=np.float32)
    wq = np.asarray(wq, dtype=np.float32)
    wk = np.asarray(wk, dtype=np.float32)
    wv = np.asarray(wv, dtype=np.float32)
    wo = np.asarray(wo, dtype=np.float32).astype(bf)
    xplanes = [planes(np.ascontiguousarray(hs[b].T), AX) for b in range(B)]

    in_maps = []
    for core in range(NCORES):
        b, g = divmod(core, G)
        wqh_, wql_ = planes(np.ascontiguousarray(wq[:, GD * g:GD * (g + 1)]), AW)
        wkh_, wkl_ = planes(np.ascontiguousarray(wk[:, D * g:D * (g + 1)]), AW)
        wvh_, wvl_ = planes(np.ascontiguousarray(wv[:, D * g:D * (g + 1)]), AW)
        in_maps.append({
            "xh": xplanes[b][0].reshape(NE, 128, S),
            "xl": xplanes[b][1].reshape(NE, 128, S),
            "wqh": wqh_.reshape(NE, 128, GD),
            "wql": wql_.reshape(NE, 128, GD),
            "wkh": wkh_.reshape(NE, 128, D),
            "wkl": wkl_.reshape(NE, 128, D),
            "wvh": wvh_.reshape(NE, 128, D),
            "wvl": wvl_.reshape(NE, 128, D),
            "wob": np.ascontiguousarray(
                wo[GD * g:GD * (g + 1), :]).reshape(G, 128, E),
            "cosT": cosT,
            "sinT": sinT,
            "rotP": rotP,
            "ident": ident,
            "onesb": onesb,
        })

    res = run_bass_kernel_spmd(_NC, in_maps, list(range(NCORES)))
    LAST_RESULTS = res
    outs = [np.asarray(res.results[i]["out"], dtype=np.float32)
            for i in range(NCORES)]
    full = np.stack([sum(outs[b * G:(b + 1) * G]) for b in range(B)], axis=0)
    return (full / PSC).astype(np.float32)


# revision 44
# speedup vs baseline: 1.3064x; 1.0506x over previous
"""GQA (16 q-heads / 4 kv-heads, D=128, S=2048, E=2048, B=2) on 8 trn2 cores.

Sharding: core = 4*b + g  (b in {0,1} batch, g in {0..3} kv-head group).
Each core computes its batch's 4 query heads (one kv group) end-to-end and
the host sums the 4 partial o_proj outputs per batch.

v3 (single interleaved program, engines balanced, DMA-consolidated):
  - Blocked DRAM layouts + AP.transpose give one DMA per weight tensor and
    4 DMAs per x position-chunk (~65 DMAs total vs ~250).
  - Phase A per 512-position chunk: K proj+RoPE, Q(h0) proj+RoPE, V proj +
    PE-transpose to natural bf16, Q(h1..h3) proj+RoPE.  RoPE rotate-half is
    a signed-permutation matmul on PE (no SBUF swap DMAs); cos/sin mults
    split DVE/DVE, add on Pool.
  - Attention in four 512-wide query chunks: scoresT per sk-tile in f32r,
    Exp on Act straight from PSUM to bf16 SBUF, bf16 AV matmuls (V-natural
    stationary), softmax denominator via two bf16 chain-adds (DVE + Pool),
    ones-matmul partition reduce, reciprocal, Pool partition_broadcast,
    DVE normalize multiply into f32r ot.
  - o_proj of chunk c interleaved into attention of chunk c+1; stores
    staged through SBUF [128,1024] tiles via DVE/Act copies.
"""

import numpy as np
import ml_dtypes

import concourse.bass as bass
import concourse.bacc as bacc
import concourse.mybir as mybir
import concourse.tile as tile
from concourse.bass_utils import run_bass_kernel_spmd

B, S, E = 2, 2048, 2048
H, HKV, D = 16, 4, 128
G = H // HKV          # 4 query heads per kv group
GD = G * D            # 512 channels per group
NCORES = 8
SCALE = 1.0 / float(np.sqrt(D))
ROPE_BASE = 10000.0
AX = 16.0             # fp8 plane scale for x
AW = 64.0             # fp8 plane scale for wq/wk/wv/wo
PSC = AX * AW         # q/k/v come out scaled by PSC
SCALE_EFF = SCALE / (PSC * PSC)   # folds the q*k scale into exp
AO = 16.0             # fp8 plane scale for the normalized attention output
# the softmax reduce uses (PSC/AO)-valued "ones", so ot = AO * attn_out and
# the o_proj result comes out scaled by AO*AW = PSC; the host divides once.
RED = PSC / AO

NE = E // 128         # 16 e-blocks (contraction for projections)
NC4 = S // 512        # 4 position chunks of 512 (projection granularity)
NST = S // 128        # 16 sk-tiles of 128
CHS = [512, 1024, 512]          # attention query-chunk widths
COFF = [0, 512, 1536]           # their offsets
CHMAX = 1024

F32 = mybir.dt.float32
F32R = mybir.dt.float32r
BF16 = mybir.dt.bfloat16
FP8 = mybir.dt.float8e4
DR = mybir.MatmulPerfMode.DoubleRow
AF = mybir.ActivationFunctionType
OP = mybir.AluOpType


def _r(ap):
    return ap.bitcast(F32R)


def _emit(nc, tc, xh, xl, wqh, wql, wkh, wkl, wvh, wvl, woh, wol, cosT, sinT, rotP, ident, onesb, out):
    from contextlib import ExitStack
    es = ExitStack()
    with es:
        cpool = es.enter_context(tc.tile_pool(name="const", bufs=1))
        wopool = es.enter_context(tc.tile_pool(name="wo", bufs=2))
        xpool = es.enter_context(tc.tile_pool(name="xs", bufs=16))
        rpool = es.enter_context(tc.tile_pool(name="rope", bufs=2))
        etpool = es.enter_context(tc.tile_pool(name="et", bufs=8))
        bcspool = es.enter_context(tc.tile_pool(name="bcs", bufs=2))
        dnpool = es.enter_context(tc.tile_pool(name="dn", bufs=2))
        rcpool = es.enter_context(tc.tile_pool(name="rc", bufs=2))
        otpool = es.enter_context(tc.tile_pool(name="ot", bufs=2))
        ostgpool = es.enter_context(tc.tile_pool(name="ostg", bufs=4))
        vtpool = es.enter_context(tc.tile_pool(name="vt", bufs=2))
        pssc = es.enter_context(
            tc.tile_pool(name="pssc", bufs=2, space=bass.MemorySpace.PSUM))
        psav = es.enter_context(
            tc.tile_pool(name="psav", bufs=1, space=bass.MemorySpace.PSUM))
        psmx = es.enter_context(
            tc.tile_pool(name="psmx", bufs=2, space=bass.MemorySpace.PSUM))

        # ---- small constants (rp needed first; the rest load after the
        # first x chunk, off the critical path to the first K matmul) ----
        id_sb = cpool.tile([128, 128], F32, tag="id")
        ones_sb = cpool.tile([128, 128], BF16, tag="ones")
        rp_sb = cpool.tile([128, 128], BF16, tag="rp")
        cos_sb = cpool.tile([D, S], BF16, tag="cos")
        sin_sb = cpool.tile([D, S], BF16, tag="sin")
        nc.sync.dma_start(out=rp_sb[:], in_=rotP.ap())

        # ---- consolidated weight loads (fp8 hi/lo double-quant planes);
        # wq/wv DMAs are issued after the first x chunk so the critical path
        # to the first K matmul is short
        wk_t = [cpool.tile([128, NE, D], FP8, tag=f"wkt{i}", name=f"wkt{i}")
                for i in range(2)]
        for i, t in enumerate((wkh, wkl)):
            nc.sync.dma_start(out=wk_t[i][:], in_=t.ap().transpose([1, 0, 2]))
        wq_t = [cpool.tile([128, NE, GD], FP8, tag=f"wqt{i}", name=f"wqt{i}")
                for i in range(2)]
        wv_t = [cpool.tile([128, NE, D], FP8, tag=f"wvt{i}", name=f"wvt{i}")
                for i in range(2)]

        # ---- persistent activations (bf16: same PE rate, half SBUF) ----
        kt = cpool.tile([D, S], BF16, tag="kt")
        qt = [cpool.tile([D, S], BF16, tag=f"qt{h}", name=f"qt{h}")
              for h in range(G)]
        vn = cpool.tile([128, NST, D], BF16, tag="vn")

        def rope(dst, ps, sl, in_attn=False):
            # rotate_half as a signed-permutation matmul; then
            # dst = q*cos + rot(q)*sin.  During phase A the qraw copy runs on
            # the idle Act engine and rot borrows an idle scores-pool PSUM
            # slot; inside attention windows both would collide with exp /
            # scores, so qraw moves to DVE and rot stays in psmx.
            qraw = rpool.tile([128, 512], BF16, tag="qraw")
            tmc = rpool.tile([128, 512], F32, tag="tmc")
            t2 = rpool.tile([128, 512], F32, tag="t2")
            if in_attn:
                nc.vector.tensor_copy(qraw[:], ps[:])
                rot = psmx.tile([128, 512], F32, tag="mx", name="rot")
            else:
                nc.scalar.copy(qraw[:], ps[:])
                rot = pssc.tile([128, 512], F32, tag="sc", name="rot")
            nc.tensor.matmul(rot[:], rp_sb[:], qraw[:], start=True, stop=True)
            nc.vector.tensor_tensor(tmc[:], qraw[:], cos_sb[:, sl], OP.mult)
            nc.vector.tensor_tensor(t2[:], rot[:], sin_sb[:, sl], OP.mult)
            nc.gpsimd.tensor_tensor(dst, tmc[:], t2[:], OP.add)

        def load_x(c4):
            sl = slice(c4 * 512, (c4 + 1) * 512)
            xsl = []
            for i, t in enumerate((xh, xl)):
                tiles = [xpool.tile([128, 4, 512], FP8, tag="xs",
                                    name=f"xs{c4}_{i}_{jj}")
                         for jj in range(4)]
                for jj in range(4):
                    nc.sync.dma_start(
                        out=tiles[jj][:],
                        in_=t.ap()[jj * 4:(jj + 1) * 4, :, sl]
                            .transpose([1, 0, 2]))
                xsl.append(tiles)
            return xsl

        PLANES = ((0, 0), (0, 1), (1, 0))   # (w_plane, x_plane): HH, HL, LH

        def proj_mms(ps, wt, cslice, xsl):
            mms = []
            for i, (wi, xi) in enumerate(PLANES):
                for p in range(NE // 2):
                    mms.append((wt[wi][:, 2 * p:2 * p + 2, cslice],
                                xsl[xi][p // 2][:, 2 * (p % 2):2 * (p % 2) + 2, :]))
            return mms

        def proj(wt, cslice, xsl):
            ps = psmx.tile([128, 512], F32, tag="mx", name="ps")
            mms = proj_mms(ps, wt, cslice, xsl)
            for i, (wa, xa) in enumerate(mms):
                nc.tensor.matmul(ps[:], wa, xa, perf_mode=DR,
                                 start=(i == 0), stop=(i == len(mms) - 1))
            return ps

        def qproj(h, c4, xsl):
            sl = slice(c4 * 512, (c4 + 1) * 512)
            ps = proj(wq_t, slice(h * D, (h + 1) * D), xsl)
            rope(qt[h][:, sl], ps, sl)

        # ============ phase A: K/V for all chunks, Q for chunk 0 ============
        # Remaining Q projections are interleaved into the act-gated
        # attention windows below (x slices reloaded through the same pool).
        for c4 in range(NC4):
            sl = slice(c4 * 512, (c4 + 1) * 512)
            xsl = load_x(c4)
            if c4 == 0:
                nc.sync.dma_start(out=cos_sb[:], in_=cosT.ap())
                nc.sync.dma_start(out=sin_sb[:], in_=sinT.ap())
                nc.sync.dma_start(out=id_sb[:], in_=ident.ap())
                nc.sync.dma_start(out=ones_sb[:], in_=onesb.ap())
                for i, t in enumerate((wqh, wql)):
                    for jj in range(2):
                        nc.sync.dma_start(
                            out=wq_t[i][:, jj * 8:(jj + 1) * 8, :],
                            in_=t.ap()[jj * 8:(jj + 1) * 8, :, :]
                                .transpose([1, 0, 2]))
                for i, t in enumerate((wvh, wvl)):
                    nc.sync.dma_start(out=wv_t[i][:],
                                      in_=t.ap().transpose([1, 0, 2]))
            # K
            ps = proj(wk_t, slice(0, D), xsl)
            rope(kt[:, sl], ps, sl)
            # V -> natural bf16 via PE transpose
            ps = proj(wv_t, slice(0, D), xsl)
            vt = vtpool.tile([D, 512], F32, tag="vt")
            nc.scalar.copy(vt[:], ps[:])
            for tt in range(4):
                trp = psmx.tile([128, 128], F32, tag="mx", name="trp")
                nc.tensor.transpose(trp[:], vt[:, tt * 128:(tt + 1) * 128],
                                    id_sb[:])
                nc.vector.tensor_copy(vn[:, c4 * 4 + tt, :], trp[:])
            if c4 == 0:
                for h in range(G):
                    qproj(h, 0, xsl)

        # ================= phases B+C: attention + o_proj =================
        # Deferred PE work (remaining Q projections, previous-chunk o_proj)
        # is queued as ~1-matmul units and drained a couple per t-iteration,
        # so the Act engine never sees a multi-microsecond scores gap.
        import collections
        fill = collections.deque()

        def drain(n):
            while n > 0 and fill:
                fill.popleft()()
                n -= 1

        def enqueue_qproj(h, c4, xsl):
            sl = slice(c4 * 512, (c4 + 1) * 512)
            state = {}
            cslice = slice(h * D, (h + 1) * D)
            nmm = 3 * (NE // 2)

            def mk(j):
                def unit():
                    if j == 0:
                        state['ps'] = psmx.tile([128, 512], F32, tag="mx",
                                                name="ps")
                        state['mms'] = proj_mms(state['ps'], wq_t, cslice, xsl)
                    ps = state['ps']
                    for jj in (2 * j, 2 * j + 1):
                        wa, xa = state['mms'][jj]
                        nc.tensor.matmul(ps[:], wa, xa, perf_mode=DR,
                                         start=(jj == 0), stop=(jj == nmm - 1))
                return unit
            for j in range(nmm // 2):
                fill.append(mk(j))
            fill.append(lambda: rope(qt[h][:, sl], state['ps'], sl,
                                     in_attn=True))

        def attn_head(off, chw, h, ot_hi, ot_lo):
            """Scores/exp/AV/denominator/normalize for a chw-wide query chunk
            at offset off, head h.  Writes the AO-scaled normalized output
            into slice h of the chunk's fp8 hi/lo ot planes."""
            nhf = chw // 512
            dn = dnpool.tile([128, chw], BF16, tag="dn")
            av = psav.tile([D, chw], F32, tag="av")
            et_first = None
            for t in range(NST):
                sc = pssc.tile([128, chw], F32, tag="sc")
                for hf in range(nhf):
                    qsl = slice(off + hf * 512, off + (hf + 1) * 512)
                    nc.tensor.matmul(sc[:, hf * 512:(hf + 1) * 512],
                                     kt[:, t * 128:(t + 1) * 128],
                                     qt[h][:, qsl], start=True, stop=True)
                et = etpool.tile([128, chw], BF16, tag="et")
                nc.scalar.activation(et[:], sc[:], AF.Exp, scale=SCALE_EFF)
                for hf in range(nhf):
                    nc.tensor.matmul(av[:, hf * 512:(hf + 1) * 512],
                                     vn[:, t, :],
                                     et[:, hf * 512:(hf + 1) * 512],
                                     start=(t == 0), stop=(t == NST - 1))
                # denominator: single DVE chain, always caught up with exp
                if t == 0:
                    et_first = et
                elif t == 1:
                    nc.vector.tensor_tensor(dn[:], et_first[:], et[:], OP.add)
                else:
                    nc.vector.tensor_tensor(dn[:], dn[:], et[:], OP.add)
                if t >= 2:
                    drain(2)
            rc = rcpool.tile([1, chw], BF16, tag="rc")
            for hf in range(nhf):
                sm = psmx.tile([1, 512], F32, tag="mx", name="sm")
                nc.tensor.matmul(sm[:], ones_sb[:, 0:1],
                                 dn[:, hf * 512:(hf + 1) * 512],
                                 start=True, stop=True)
                with nc.allow_low_precision(reason="bf16 softmax denom recip"):
                    nc.vector.reciprocal(rc[:, hf * 512:(hf + 1) * 512], sm[:])
            bcs = bcspool.tile([128, chw], BF16, tag="bcs")
            nc.gpsimd.partition_broadcast(bcs[:], rc[:])
            otf = rpool.tile([D, chw], F32, tag="otf")
            nc.vector.tensor_tensor(otf[:], av[:], bcs[:], OP.mult)
            nc.scalar.copy(ot_hi[:, h, :], otf[:])
            nc.vector.tensor_tensor(ot_lo[:, h, :], otf[:], ot_hi[:, h, :],
                                    OP.subtract)

        def enqueue_oproj_pair(off, chw, eo0, oth, otl, ci, copy_eng):
            """Two adjacent eo column-groups (1024 cols of E) of o_proj for
            the chunk at offset off, via fp8 DoubleRow over (h-pair, plane):
            one [128,1024] staging tile per s-tile, one store each."""
            wots = []
            for eo in (eo0, eo0 + 1):
                wot = [wopool.tile([128, G, 512], FP8, tag=f"wo{i}",
                                   name=f"wo{ci}_{eo}_{i}") for i in range(2)]
                for i, t in enumerate((woh, wol)):
                    nc.sync.dma_start(
                        out=wot[i][:],
                        in_=t.ap()[:, :, eo * 512:(eo + 1) * 512]
                            .transpose([1, 0, 2]))
                wots.append(wot)
            for st in range(chw // 128):
                state = {}

                def mk(st, k):
                    def unit():
                        if k == 0:
                            state['ostg'] = ostgpool.tile(
                                [128, 1024], F32, tag="ostg",
                                name=f"ostg{ci}_{eo0}_{st}")
                        ostg = state['ostg']
                        op = psmx.tile([128, 512], F32, tag="mx", name="op")
                        ssl = slice(st * 128, (st + 1) * 128)
                        mms = []
                        for ota, wi in ((oth, 0), (oth, 1), (otl, 0)):
                            for i in range(2):
                                mms.append((ota[:, 2 * i:2 * i + 2, ssl],
                                            wots[k][wi][:, 2 * i:2 * i + 2, :]))
                        for i, (oa, wa) in enumerate(mms):
                            nc.tensor.matmul(op[:], oa, wa, perf_mode=DR,
                                             start=(i == 0),
                                             stop=(i == len(mms) - 1))
                        dst = ostg[:, k * 512:(k + 1) * 512]
                        if copy_eng == 'act':
                            nc.scalar.copy(dst, op[:])
                        else:
                            nc.vector.tensor_copy(dst, op[:])
                        if k == 1:
                            nc.sync.dma_start(
                                out=out.ap()[off + st * 128:
                                             off + (st + 1) * 128,
                                             eo0 * 512:(eo0 + 2) * 512],
                                in_=ostg[:])
                    return unit
                fill.append(mk(st, 0))
                fill.append(mk(st, 1))

        def ot_planes(ci):
            chw = CHS[ci]
            hi = otpool.tile([128, G, chw], FP8, tag="oth", name=f"oth{ci}")
            lo = otpool.tile([128, G, chw], FP8, tag="otl", name=f"otl{ci}")
            return hi, lo

        # B0 (512-wide): deferred Q projections for position chunks 1 and 2
        # drain into the PE slack of the attention loop.  x slices are
        # re-loaded one stage ahead so filler matmuls never wait on DMA.
        xs1 = load_x(1)
        oth0, otl0 = ot_planes(0)
        xs2 = None
        for h in range(G):
            if h == 0:
                for hq in range(G):
                    enqueue_qproj(hq, 1, xs1)
            elif h == 1:
                xs2 = load_x(2)
            elif h == 2:
                for hq in range(G):
                    enqueue_qproj(hq, 2, xs2)
            attn_head(COFF[0], CHS[0], h, oth0, otl0)
            drain(8)
        drain(len(fill))
        # B1 (1024-wide): o_proj of chunk 0 + Q projections for chunk 3
        xs3 = load_x(3)
        oth1, otl1 = ot_planes(1)
        for h in range(G):
            if h == 0:
                enqueue_oproj_pair(COFF[0], CHS[0], 0, oth0, otl0, 0, 'dve')
                for hq in range(G):
                    enqueue_qproj(hq, 3, xs3)
                enqueue_oproj_pair(COFF[0], CHS[0], 2, oth0, otl0, 0, 'dve')
            attn_head(COFF[1], CHS[1], h, oth1, otl1)
            drain(8)
        drain(len(fill))
        # B2 (512-wide): o_proj of chunk 1, spread across all four heads
        oth2, otl2 = ot_planes(2)
        for h in range(G):
            if h == 0:
                enqueue_oproj_pair(COFF[1], CHS[1], 0, oth1, otl1, 1, 'dve')
            elif h == 2:
                enqueue_oproj_pair(COFF[1], CHS[1], 2, oth1, otl1, 1, 'dve')
            attn_head(COFF[2], CHS[2], h, oth2, otl2)
            drain(8)
        drain(len(fill))
        # tail: o_proj of chunk 2, staging copies on the now-idle Act engine
        enqueue_oproj_pair(COFF[2], CHS[2], 0, oth2, otl2, 2, 'act')
        enqueue_oproj_pair(COFF[2], CHS[2], 2, oth2, otl2, 2, 'act')
        drain(len(fill))


def _build():
    nc = bacc.Bacc("TRN2", target_bir_lowering=False, debug=False,
                   num_devices=NCORES)
    xh = nc.dram_tensor("xh", [NE, 128, S], FP8, kind="ExternalInput")
    xl = nc.dram_tensor("xl", [NE, 128, S], FP8, kind="ExternalInput")
    wqh = nc.dram_tensor("wqh", [NE, 128, GD], FP8, kind="ExternalInput")
    wql = nc.dram_tensor("wql", [NE, 128, GD], FP8, kind="ExternalInput")
    wkh = nc.dram_tensor("wkh", [NE, 128, D], FP8, kind="ExternalInput")
    wkl = nc.dram_tensor("wkl", [NE, 128, D], FP8, kind="ExternalInput")
    wvh = nc.dram_tensor("wvh", [NE, 128, D], FP8, kind="ExternalInput")
    wvl = nc.dram_tensor("wvl", [NE, 128, D], FP8, kind="ExternalInput")
    woh = nc.dram_tensor("woh", [G, 128, E], FP8, kind="ExternalInput")
    wol = nc.dram_tensor("wol", [G, 128, E], FP8, kind="ExternalInput")
    cosT = nc.dram_tensor("cosT", [D, S], BF16, kind="ExternalInput")
    sinT = nc.dram_tensor("sinT", [D, S], BF16, kind="ExternalInput")
    rotP = nc.dram_tensor("rotP", [128, 128], BF16, kind="ExternalInput")
    ident = nc.dram_tensor("ident", [128, 128], F32, kind="ExternalInput")
    onesb = nc.dram_tensor("onesb", [128, 128], BF16, kind="ExternalInput")
    out = nc.dram_tensor("out", [S, E], F32, kind="ExternalOutput")
    with tile.TileContext(nc) as tc:
        _emit(nc, tc, xh, xl, wqh, wql, wkh, wkl, wvh, wvl, woh, wol, cosT,
              sinT, rotP, ident, onesb, out)
    nc.compile()
    return nc


def _rope_tables():
    inv = 1.0 / (ROPE_BASE ** (np.arange(0, D, 2, dtype=np.float64) / D))
    t = np.arange(S, dtype=np.float64)
    freqs = t[:, None] * inv[None, :]                    # [S, D/2]
    emb = np.concatenate([freqs, freqs], axis=-1)        # [S, D]
    cosT = np.cos(emb).T.astype(ml_dtypes.bfloat16)      # [D, S]
    sinT = np.sin(emb).T.astype(ml_dtypes.bfloat16)
    return np.ascontiguousarray(cosT), np.ascontiguousarray(sinT)


def _rot_perm():
    # rot(q)[d] = -q[d+64] for d<64, +q[d-64] for d>=64, as a stationary
    # matmul operand: rot = P^T @ q with P[k, m] below.
    p = np.zeros((128, 128), dtype=ml_dtypes.bfloat16)
    for d in range(64):
        p[d + 64, d] = -1.0
        p[d, d + 64] = 1.0
    return p


_NC = None
LAST_RESULTS = None


def kernel(hidden_states, wq, wk, wv, wo):
    global _NC, LAST_RESULTS
    if _NC is None:
        _NC = _build()
    cosT, sinT = _rope_tables()
    ident = np.eye(128, dtype=np.float32)
    onesb = np.full((128, 128), RED, dtype=ml_dtypes.bfloat16)
    rotP = _rot_perm()
    bf = ml_dtypes.bfloat16
    f8 = ml_dtypes.float8_e4m3

    def planes(a, scale):
        hi = (scale * a).astype(f8)
        lo = (scale * a - hi.astype(np.float32)).astype(f8)
        return hi, lo

    hs = np.asarray(hidden_states, dtype=np.float32)
    wq = np.asarray(wq, dtype=np.float32)
    wk = np.asarray(wk, dtype=np.float32)
    wv = np.asarray(wv, dtype=np.float32)
    wo = np.asarray(wo, dtype=np.float32)
    xplanes = [planes(np.ascontiguousarray(hs[b].T), AX) for b in range(B)]

    in_maps = []
    for core in range(NCORES):
        b, g = divmod(core, G)
        wqh_, wql_ = planes(np.ascontiguousarray(wq[:, GD * g:GD * (g + 1)]), AW)
        wkh_, wkl_ = planes(np.ascontiguousarray(wk[:, D * g:D * (g + 1)]), AW)
        wvh_, wvl_ = planes(np.ascontiguousarray(wv[:, D * g:D * (g + 1)]), AW)
        woh_, wol_ = planes(np.ascontiguousarray(wo[GD * g:GD * (g + 1), :]), AW)
        in_maps.append({
            "xh": xplanes[b][0].reshape(NE, 128, S),
            "xl": xplanes[b][1].reshape(NE, 128, S),
            "wqh": wqh_.reshape(NE, 128, GD),
            "wql": wql_.reshape(NE, 128, GD),
            "wkh": wkh_.reshape(NE, 128, D),
            "wkl": wkl_.reshape(NE, 128, D),
            "wvh": wvh_.reshape(NE, 128, D),
            "wvl": wvl_.reshape(NE, 128, D),
            "woh": woh_.reshape(G, 128, E),
            "wol": wol_.reshape(G, 128, E),
            "cosT": cosT,
            "sinT": sinT,
            "rotP": rotP,
            "ident": ident,
            "onesb": onesb,
        })

    res = run_bass_kernel_spmd(_NC, in_maps, list(range(NCORES)))
    LAST_RESULTS = res
    outs = [np.asarray(res.results[i]["out"], dtype=np.float32)
            for i in range(NCORES)]
    full = np.stack([sum(outs[b * G:(b + 1) * G]) for b in range(B)], axis=0)
    return (full / PSC).astype(np.float32)
